# revision 19
# baseline (speedup 1.0000x reference)
"""Distributed Bass/Trainium2 kernel for nn_AreaGNN: 3x SAGEConv(mean) +
global BatchNorm + ReLU, per-graph mean/max pooling, 3-layer MLP head.
SPMD across 8 NeuronCores; takes FULL inputs, returns FULL output [G].

v5:
- dma_gather over 4 SWDGE queues (disjoint Q7 descgen pairs, ~4x).
- One-hot S blocks generated on-chip (DVE is_equal*invdeg) from a tiny
  per-block table; first NCACHE chunks cached in SBUF across layers.
- Aggregation matmul: out[feat, dst] = msgs^T @ S (feature-major agg, no
  dense-phase transposes); agg staged f16.
- Dense phase (z matmuls + BN stat accumulation) interleaved into the chunk
  loop per-tile as each tile's aggregation completes.
- Halo exchange split into TWO region AllGathers (shard rows 0:3072 and
  3072:6272). Gather tables are laid out region-major so region-0 chunks
  start as soon as AG0 lands while AG1 (and its transposes) overlap them.
"""
import numpy as np

N = 50000
E = 800000
D = 128
HID = 128
G = 64
G_FEAT = 32
EPS = 1e-5
NCORES = 8
NSH = N // NCORES           # 6250
NSH_PAD = 6272              # 49 * 128
NTILES = NSH_PAD // 128     # 49
RSPLIT = 3072               # region split within a shard (24 tiles | 25 tiles)
K0, K1 = RSPLIT, NSH_PAD - RSPLIT          # 3072, 3200 rows/core/region
R0, R1 = K0 * NCORES, K1 * NCORES          # 24576, 25600 table rows
BLK = 128                   # edges per S block
CBLK = 32                   # blocks per gather chunk (4096 edges)
CH = BLK * CBLK
CPW = CH // 16
NCACHE = 4                  # S chunks cached in SBUF across layers
TABLE_SHARED = True


# ---------------- host-side preprocessing -----------------------------------

def _wrap_idx(idx, ch):
    """[L] -> [L/ch, 128, ch/16] int16: element m of a chunk at (m%16, m//16),
    replicated across the eight 16-partition groups."""
    L = idx.shape[0]
    out = np.empty((L // ch, 128, ch // 16), dtype=np.int16)
    w = idx.reshape(L // ch, ch // 16, 16).transpose(0, 2, 1)
    for g in range(8):
        out[:, g * 16:(g + 1) * 16, :] = w
    return out


def _preprocess(x, edge_index, batch):
    src = np.asarray(edge_index[0], dtype=np.int64)
    dst = np.asarray(edge_index[1], dtype=np.int64)
    batch = np.asarray(batch, dtype=np.int64)

    indeg = np.bincount(dst, minlength=N)
    invdeg_all = (1.0 / np.maximum(indeg, 1.0)).astype(np.float32)

    core_of = dst // NSH
    tile_of = (dst % NSH) // 128
    # region of src within its owner shard + region-relative table index
    src_core = src // NSH
    src_off = src % NSH
    half_of = (src_off >= RSPLIT).astype(np.int64)
    src_reg_idx = np.where(half_of == 0,
                           src_core * K0 + src_off,
                           src_core * K1 + (src_off - RSPLIT))

    counts = np.zeros((NCORES, 2, NTILES), dtype=np.int64)
    buckets = {}
    for c in range(NCORES):
        mc = core_of == c
        for h in range(2):
            mh = mc & (half_of == h)
            for t in range(NTILES):
                m = mh & (tile_of == t)
                g = src_reg_idx[m]
                d = (dst[m] % NSH) % 128        # dst within tile
                w = invdeg_all[dst[m]]
                buckets[(c, h, t)] = (g, d, w)
                counts[c, h, t] = len(g)

    # global block schedule: both halves get >= 1 block per tile (pass-A copy
    # initializes agg; pass-B stop triggers the interleaved dense step)
    nblk = np.ceil(counts.max(axis=0) / BLK).astype(np.int64)  # [2, NTILES]
    nblk = np.maximum(nblk, 1)
    extra = [0, 0]
    for h in range(2):
        tot = int(nblk[h].sum())
        extra[h] = (-tot) % CBLK
    sched = []   # list of (h, t) per block, in execution order
    for h in range(2):
        for t in range(NTILES):
            sched += [(h, t)] * int(nblk[h, t])
        sched += [(h, NTILES - 1)] * extra[h]
    nblk_tot = len(sched)
    nchunks = nblk_tot // CBLK
    assert nchunks * CBLK == nblk_tot
    chunk_half = [sched[k * CBLK][0] for k in range(nchunks)]
    for k in range(nchunks):
        assert all(sched[k * CBLK + j][0] == chunk_half[k] for j in range(CBLK))

    # per-chunk gather pieces [(col offset in chunk, num_idxs)]
    run_start = {}
    b0 = 0
    for h in range(2):
        for t in range(NTILES):
            nb = int(nblk[h, t]) + (extra[h] if t == NTILES - 1 else 0)
            run_start[(h, t)] = (b0, nb)
            b0 += nb
    r16 = {k: min(-(-int(counts[:, k[0], k[1]].max()) // 16) * 16,
                  run_start[k][1] * BLK)
           for k in run_start}
    gather_pieces = []
    for k in range(nchunks):
        c0, c1 = k * CBLK * BLK, (k + 1) * CBLK * BLK
        iv = []
        for (h, t), (rb, nb) in run_start.items():
            if h != chunk_half[k]:
                continue
            s0, s1 = rb * BLK, rb * BLK + r16[(h, t)]
            a, b = max(s0, c0), min(s1, c1)
            if a < b:
                iv.append((a - c0, b - c0))
        pieces = []
        for q in range(0, CBLK * BLK, 1024):
            if any(a < q + 1024 and b > q for a, b in iv):
                pieces.append((q, 1024))
        gather_pieces.append(pieces)

    # per-core gather idx + per-block [dstidx, invdeg] following the schedule
    # (also original src node ids per slot, for the layer-0 host pregather)
    gidx_cores, div_cores, srcids_cores, S_cores = [], [], [], []
    src_orig = {}
    for c in range(NCORES):
        mc = core_of == c
        for h in range(2):
            mh = mc & (half_of == h)
            for t in range(NTILES):
                m = mh & (tile_of == t)
                src_orig[(c, h, t)] = src[m]
    for c in range(NCORES):
        gi = np.zeros(nblk_tot * BLK, dtype=np.int64)
        sid = np.zeros(nblk_tot * BLK, dtype=np.int64)
        div = np.zeros((nblk_tot * BLK, 2), dtype=np.float32)
        b0 = 0
        for h in range(2):
            for t in range(NTILES):
                nb = int(nblk[h, t]) + (extra[h] if t == NTILES - 1 else 0)
                g, d, w = buckets[(c, h, t)]
                n = len(g)
                gi[b0 * BLK: b0 * BLK + n] = g
                sid[b0 * BLK: b0 * BLK + n] = src_orig[(c, h, t)]
                div[b0 * BLK: b0 * BLK + n, 0] = d.astype(np.float32)
                div[b0 * BLK: b0 * BLK + n, 1] = w.astype(np.float32)
                b0 += nb
        assert b0 == nblk_tot
        gidx_cores.append(_wrap_idx(gi.astype(np.int16), CH))
        srcids_cores.append(sid)
        div_cores.append(np.ascontiguousarray(
            div.reshape(nblk_tot, BLK, 2).transpose(1, 0, 2)))
        Sm = np.zeros((nblk_tot * BLK, 128), dtype=np.float16)
        slots = np.arange(nblk_tot * BLK)
        Sm[slots, div[:, 0].astype(np.int64)] = div[:, 1].astype(np.float16)
        S_cores.append(np.ascontiguousarray(
            Sm.reshape(nblk_tot, BLK, 128).transpose(1, 0, 2)
            .reshape(BLK, nblk_tot * 128)))

    # last pass-B block index per tile (the interleaved dense trigger)
    pbstop = {}
    for b, (h, t) in enumerate(sched):
        if h == 1:
            pbstop[t] = b
    dense_after = {b: t for t, b in pbstop.items()}

    cnt_g = np.bincount(batch, minlength=G)
    inv_cnt = (1.0 / np.maximum(cnt_g, 1.0)).astype(np.float32)

    P = []
    for c in range(NCORES):
        p = np.zeros((NSH_PAD, G), dtype=np.float32)
        b = batch[c * NSH:(c + 1) * NSH]
        p[np.arange(NSH), b] = inv_cnt[b]
        P.append(p)

    NG, Smax = 0, 0
    groups_c = []
    for c in range(NCORES):
        b = batch[c * NSH:(c + 1) * NSH]
        glo, ghi = int(b.min()), int(b.max())
        groups = [(g, np.where(b == g)[0]) for g in range(glo, ghi + 1)]
        groups_c.append((glo, groups))
        NG = max(NG, ghi - glo + 1)
        Smax = max(Smax, max(len(gr) for _, gr in groups))
    S_slot = ((Smax + 127) // 128) * 128
    slot, route = [], []
    for c in range(NCORES):
        glo, groups = groups_c[c]
        sm = np.full(NG * S_slot, NSH, dtype=np.int64)   # NSH = zero dummy row
        R = np.zeros((NG, G), dtype=np.float32)
        for g, gr in groups:
            r = g - glo
            sm[r * S_slot:r * S_slot + len(gr)] = gr
            R[r, g] = 1.0
        slot.append(_wrap_idx(sm.astype(np.int16), NG * S_slot)[0])
        route.append(R)

    return dict(nblk=nblk, extra=extra, sched=sched, nblk_tot=nblk_tot,
                nchunks=nchunks, chunk_half=chunk_half,
                gather_pieces=gather_pieces, dense_after=dense_after,
                gidx=gidx_cores, div=div_cores, srcids=srcids_cores,
                S=S_cores, P=P,
                slot=slot, route=route, S_slot=S_slot, NG=NG)


# ---------------- device kernel builder --------------------------------------

def _build(nc, pre):
    import concourse.mybir as mybir
    import concourse.tile as tile

    f32 = mybir.dt.float32
    f16 = mybir.dt.float16
    i16 = mybir.dt.int16
    NCH = pre['nchunks']
    NBLK_TOT = pre['nblk_tot']
    NG, S_slot = pre['NG'], pre['S_slot']
    NSLOT = NG * S_slot
    sched = pre['sched']
    dense_after = pre['dense_after']

    is_start = [True] * NBLK_TOT
    is_stop = [True] * NBLK_TOT
    for b in range(NBLK_TOT):
        if b > 0 and sched[b] == sched[b - 1]:
            is_start[b] = False
        if b < NBLK_TOT - 1 and sched[b] == sched[b + 1]:
            is_stop[b] = False
    # block position within its run + run length (for A/B psum splitting)
    run_pos = [0] * NBLK_TOT
    run_len = [0] * NBLK_TOT
    p = 0
    for b in range(NBLK_TOT):
        run_pos[b] = p
        p = 0 if is_stop[b] else p + 1
    L = 0
    for b in range(NBLK_TOT - 1, -1, -1):
        if is_stop[b]:
            L = run_pos[b] + 1
        run_len[b] = L

    qctr = [0]

    def next_q():
        q = [1, 2, 3, 1, 2, 3, 0][qctr[0] % 7]
        qctr[0] += 1
        return q

    # ---- I/O ----
    msgs0_d = nc.dram_tensor("msgs0", [NCH, 128, CBLK, D], f16,
                             kind="ExternalInput")
    S_d = nc.dram_tensor("S", [BLK, NBLK_TOT * 128], f16,
                         kind="ExternalInput")
    xownT = nc.dram_tensor("xownT", [128, NSH_PAD], f16, kind="ExternalInput")
    gidx_d = nc.dram_tensor("gidx", [NCH, 128, CPW], i16, kind="ExternalInput")
    div_d = nc.dram_tensor("div", [128, NBLK_TOT, 2], f32,
                           kind="ExternalInput")
    iota_d = nc.dram_tensor("iota", [128, 128], f16, kind="ExternalInput")
    slot_d = nc.dram_tensor("slot", [128, NSLOT // 16], i16, kind="ExternalInput")
    P_d = nc.dram_tensor("P", [NSH_PAD, G], f32, kind="ExternalInput")
    route_d = nc.dram_tensor("route", [NG, G], f32, kind="ExternalInput")
    gfT_d = nc.dram_tensor("gfT", [G_FEAT, G], f32, kind="ExternalInput")
    ident_d = nc.dram_tensor("ident", [128, 128], f32, kind="ExternalInput")
    Wl_d = [nc.dram_tensor(f"Wl{i}", [D, HID], f16, kind="ExternalInput")
            for i in range(3)]
    Wr_d = [nc.dram_tensor(f"Wr{i}", [D, HID], f16, kind="ExternalInput")
            for i in range(3)]
    gb_d = [nc.dram_tensor(f"gb{i}", [HID, 2], f32, kind="ExternalInput")
            for i in range(3)]
    W1_d = nc.dram_tensor("W1", [2 * HID + G_FEAT, HID], f32, kind="ExternalInput")
    W2_d = nc.dram_tensor("W2", [HID, HID // 2], f32, kind="ExternalInput")
    W3_d = nc.dram_tensor("W3", [HID // 2, 1], f32, kind="ExternalInput")
    bT_d = nc.dram_tensor("bT", [HID, 3], f32, kind="ExternalInput")

    out_d = nc.dram_tensor("out", [G], f32, kind="ExternalOutput")

    rg = [list(range(NCORES))]

    with tile.TileContext(nc) as tc:
        with (
            tc.tile_pool(name="sb", bufs=3) as sb,
            tc.tile_pool(name="big", bufs=2) as bigp,       # zT f16 ring
            tc.tile_pool(name="agg", bufs=1) as aggp,       # agg f16
            tc.tile_pool(name="big1", bufs=1) as big1,      # gmax/allp
            tc.tile_pool(name="msg", bufs=4) as msgp,
            tc.tile_pool(name="scache", bufs=1) as scp,
            tc.tile_pool(name="idx", bufs=4) as idxp,
            tc.tile_pool(name="cst", bufs=1) as cst,
            tc.tile_pool(name="ps", bufs=1, space="PSUM") as ps,
            tc.tile_pool(name="pst", bufs=2, space="PSUM") as pst,
            tc.tile_pool(name="psa", bufs=3, space="PSUM") as psa,
            tc.tile_pool(name="psm", bufs=1, space="PSUM") as psm,
            tc.tile_pool(name="dram", bufs=1, space="DRAM") as dram,
        ):
            # ---- DRAM scratch ----
            hbounce = [dram.tile([NSH_PAD, D], f16, tag=f"hb{i}", name=f"hb{i}")
                       for i in range(2)]
            tblA = [dram.tile([R0, D], f16, tag=f"tblA{i}", name=f"tblA{i}",
                              addr_space="Shared" if TABLE_SHARED else "Local")
                    for i in range(2)]
            tblB = [dram.tile([R1, D], f16, tag=f"tblB{i}", name=f"tblB{i}",
                              addr_space="Shared" if TABLE_SHARED else "Local")
                    for i in range(2)]
            h3bf = dram.tile([NSH + 128, D], f16, tag="h3bf")
            stats_in = [dram.tile([D, 2], f32, tag=f"stats_in{i}",
                                  name=f"stats_in{i}") for i in range(3)]
            stats_out = [dram.tile([NCORES * D, 2], f32, tag=f"stats_out{i}",
                                   name=f"stats_out{i}", addr_space="Shared")
                         for i in range(3)]
            pool_in = dram.tile([D, 2 * G], f32, tag="pool_in")
            pool_out = dram.tile([NCORES * D, 2 * G], f32, tag="pool_out",
                                 addr_space="Shared")

            def load_const(src_ap, rows, cols, name, dt=f32):
                t = cst.tile([rows, cols], dt, tag=name)
                nc.sync.dma_start(out=t[:, :], in_=src_ap)
                return t

            ident_sb = load_const(ident_d[:, :], 128, 128, "ident")
            ident16_sb = cst.tile([128, 128], f16, tag="ident16")
            nc.vector.tensor_copy(ident16_sb[:, :], ident_sb[:, :])
            iota_sb = load_const(iota_d[:, :], 128, 128, "iota", f16)
            div_sb = cst.tile([128, NBLK_TOT, 2], f32, tag="div")
            nc.sync.dma_start(out=div_sb[:, :, :], in_=div_d[:, :, :])
            xT_sb = cst.tile([128, NSH_PAD], f16, tag="xT")
            nc.sync.dma_start(out=xT_sb[:, :], in_=xownT[:, :])

            scache = [scp.tile([128, CBLK, 128], f16, tag=f"Sc{k}",
                               name=f"Sc{k}")
                      for k in range(NCACHE)]

            hT_prev = xT_sb

            for li in range(3):
                Wl_sb = load_const(Wl_d[li][:, :], D, HID, f"Wl{li}", f16)
                Wr_sb = load_const(Wr_d[li][:, :], D, HID, f"Wr{li}", f16)
                gb_sb = load_const(gb_d[li][:, :], HID, 2, f"gb{li}")

                agg_sb = aggp.tile([128, NSH_PAD], f16, tag="agg")
                zT = bigp.tile([128, NSH_PAD], f16, tag="zT")
                zsum = sb.tile([128, NTILES], f32, tag="zsum")
                zsq = sb.tile([128, NTILES], f32, tag="zsq")

                # ---- chunk loop: gather + S + agg matmuls + inline dense ----
                acc_ps = None
                for k in range(NCH):
                    h = pre['chunk_half'][k]
                    msgs = msgp.tile([128, CBLK, D], f16, tag="msgs")
                    if li == 0:
                        nc.sync.dma_start(out=msgs[:, :, :],
                                          in_=msgs0_d[k, :, :, :])
                    else:
                        src_tab = (tblA[(li - 1) % 2][:, :] if h == 0
                                   else tblB[(li - 1) % 2][:, :])
                        gi = idxp.tile([128, CPW], i16, tag="gi")
                        nc.sync.dma_start(out=gi[:], in_=gidx_d[k, :, :])
                        for off, n in pre['gather_pieces'][k]:
                            nc.gpsimd.dma_gather(
                                msgs[:, off // 128:
                                     off // 128 + (n + 127) // 128,
                                     :], src_tab,
                                gi[:, off // 16:off // 16 + n // 16], n, n, D,
                                queue_num=next_q())
                    if k < NCACHE:
                        S_sb = scache[k]
                    else:
                        S_sb = msgp.tile([128, CBLK, 128], f16, tag="Ssb")
                    if li == 0:
                        nc.sync.dma_start(
                            out=S_sb[:, :, :],
                            in_=S_d[:, k * CBLK * 128:(k + 1) * CBLK * 128]
                            .rearrange("p (j d) -> p j d", d=128))
                    for j in range(CBLK):
                        b = k * CBLK + j
                        h_b, t_b = sched[b]
                        if li > 0 and k >= NCACHE:
                            # S[slot, dstcol] = (iota == dstidx[slot])*invdeg
                            nc.vector.tensor_scalar(
                                S_sb[:, j, :], iota_sb[:, :],
                                div_sb[:, b, 0:1], div_sb[:, b, 1:2],
                                mybir.AluOpType.is_equal,
                                mybir.AluOpType.mult)
                        if is_start[b]:
                            acc_ps = psa.tile([128, D], f32, tag="accp")
                        nc.tensor.matmul(acc_ps[:, :], msgs[:, j, :],
                                         S_sb[:, j, :],
                                         start=is_start[b], stop=is_stop[b])
                        if is_stop[b]:
                            sl = agg_sb[:, t_b * 128:(t_b + 1) * 128]
                            if h_b == 0:
                                nc.scalar.copy(sl, acc_ps[:, :])
                            else:
                                nc.vector.tensor_add(sl, sl, acc_ps[:, :])
                            t_d = dense_after.get(b)
                            if t_d is not None:
                                z_ps = ps.tile([128, D], f32, tag="z")
                                nc.tensor.matmul(
                                    z_ps[:, :], Wl_sb[:, :],
                                    agg_sb[:, t_d * 128:(t_d + 1) * 128],
                                    start=True, stop=False)
                                nc.tensor.matmul(
                                    z_ps[:, :], Wr_sb[:, :],
                                    hT_prev[:, t_d * 128:(t_d + 1) * 128],
                                    start=False, stop=True)
                                nc.scalar.activation(
                                    zT[:, t_d * 128:(t_d + 1) * 128],
                                    z_ps[:, :],
                                    mybir.ActivationFunctionType.Copy,
                                    accum_out=zsum[:, t_d:t_d + 1])
                                sq_scr = sb.tile([128, D], f32, tag="sqscr")
                                nc.scalar.activation(
                                    sq_scr[:, :],
                                    zT[:, t_d * 128:(t_d + 1) * 128],
                                    mybir.ActivationFunctionType.Square,
                                    accum_out=zsq[:, t_d:t_d + 1])

                # ---- BN stats exchange + scale/shift ----
                stat_sb = sb.tile([128, 2], f32, tag="stat")
                nc.vector.tensor_reduce(stat_sb[:, 0:1], zsum[:, :],
                                        mybir.AxisListType.X,
                                        mybir.AluOpType.add)
                nc.vector.tensor_reduce(stat_sb[:, 1:2], zsq[:, :],
                                        mybir.AxisListType.X,
                                        mybir.AluOpType.add)
                nc.sync.dma_start(out=stats_in[li][:, :], in_=stat_sb[:, :])
                nc.gpsimd.collective_compute(
                    "AllGather", mybir.AluOpType.bypass, replica_groups=rg,
                    ins=[stats_in[li].opt()], outs=[stats_out[li].opt()])
                allst = sb.tile([128, NCORES, 2], f32, tag="allst")
                nc.sync.dma_start(
                    out=allst[:, :, :],
                    in_=stats_out[li][:, :].rearrange("(c p) j -> p c j",
                                                      c=NCORES))
                tot = sb.tile([128, 2], f32, tag="tot")
                nc.vector.tensor_add(tot[:, :], allst[:, 0, :], allst[:, 1, :])
                for c in range(2, NCORES):
                    nc.vector.tensor_add(tot[:, :], tot[:, :], allst[:, c, :])
                mu = sb.tile([128, 6], f32, tag="mu")
                nc.scalar.mul(mu[:, 0:1], tot[:, 0:1], 1.0 / N)
                nc.scalar.mul(mu[:, 1:2], tot[:, 1:2], 1.0 / N)
                nc.vector.tensor_mul(mu[:, 2:3], mu[:, 0:1], mu[:, 0:1])
                nc.vector.tensor_sub(mu[:, 3:4], mu[:, 1:2], mu[:, 2:3])
                nc.vector.tensor_scalar_add(mu[:, 3:4], mu[:, 3:4], EPS)
                nc.vector.reciprocal(mu[:, 4:5], mu[:, 3:4])
                nc.scalar.sqrt(mu[:, 4:5], mu[:, 4:5])
                nc.vector.tensor_mul(mu[:, 4:5], mu[:, 4:5], gb_sb[:, 0:1])
                nc.vector.tensor_mul(mu[:, 5:6], mu[:, 0:1], mu[:, 4:5])
                nc.vector.tensor_sub(mu[:, 5:6], gb_sb[:, 1:2], mu[:, 5:6])

                # ---- relu (real cols; pads stay 0) + per-tile transpose/ship
                if li < 2:
                    hb = hbounce[li % 2]
                    for ck in range(13):
                        w = 512 if ck < 12 else NSH - 12 * 512
                        nc.scalar.activation(zT[:, ck * 512:ck * 512 + w],
                                             zT[:, ck * 512:ck * 512 + w],
                                             mybir.ActivationFunctionType.Relu,
                                             bias=mu[:, 5:6], scale=mu[:, 4:5])
                        t0, t1 = ck * 4, min(ck * 4 + 4, NTILES)
                        for t in range(t0, t1):
                            hT_ps = pst.tile([128, D], f16, tag="tp16")
                            nc.tensor.transpose(
                                hT_ps[:, :], zT[:, t * 128:(t + 1) * 128],
                                ident16_sb[:, :])
                            hbf_sb = sb.tile([128, D], f16, tag="hbf")
                            nc.vector.tensor_copy(hbf_sb[:, :], hT_ps[:, :])
                            nc.sync.dma_start(
                                out=hb[t * 128:(t + 1) * 128, :],
                                in_=hbf_sb[:, :])
                        if t1 == 24:   # region 0 shipped -> AG0
                            nc.gpsimd.collective_compute(
                                "AllGather", mybir.AluOpType.bypass,
                                replica_groups=rg,
                                ins=[hb[0:RSPLIT, :].opt()],
                                outs=[tblA[li % 2].opt()])
                    nc.gpsimd.collective_compute(
                        "AllGather", mybir.AluOpType.bypass, replica_groups=rg,
                        ins=[hb[RSPLIT:NSH_PAD, :].opt()],
                        outs=[tblB[li % 2].opt()])
                else:
                    meanT_ps = psm.tile([128, G], f32, tag="meanT")
                    for ck in range(13):
                        w = 512 if ck < 12 else NSH - 12 * 512
                        nc.scalar.activation(zT[:, ck * 512:ck * 512 + w],
                                             zT[:, ck * 512:ck * 512 + w],
                                             mybir.ActivationFunctionType.Relu,
                                             bias=mu[:, 5:6], scale=mu[:, 4:5])
                        t0, t1 = ck * 4, min(ck * 4 + 4, NTILES)
                        for t in range(t0, t1):
                            hT_ps = pst.tile([128, D], f16, tag="tp16")
                            nc.tensor.transpose(
                                hT_ps[:, :], zT[:, t * 128:(t + 1) * 128],
                                ident16_sb[:, :])
                            h3_sb = sb.tile([128, D], f32, tag="h3")
                            nc.vector.tensor_copy(h3_sb[:, :], hT_ps[:, :])
                            P_sb = sb.tile([128, G], f32, tag="P")
                            nc.sync.dma_start(
                                out=P_sb[:, :],
                                in_=P_d[t * 128:(t + 1) * 128, :])
                            nc.tensor.matmul(meanT_ps[:, :], h3_sb[:, :],
                                             P_sb[:, :],
                                             start=(t == 0),
                                             stop=(t == NTILES - 1))
                            hbf_sb = sb.tile([128, D], f16, tag="hbf")
                            nc.vector.tensor_copy(hbf_sb[:, :], h3_sb[:, :])
                            nc.sync.dma_start(
                                out=h3bf[t * 128:(t + 1) * 128, :],
                                in_=hbf_sb[:, :])
                    zrow = sb.tile([1, D], f16, tag="zrow")
                    nc.vector.memset(zrow[:, :], 0.0)
                    nc.sync.dma_start(out=h3bf[NSH:NSH + 1, :], in_=zrow[:, :])
                hT_prev = zT

            # ---- max pool: transpose-gather + segmented max + route ----
            slot_sb = cst.tile([128, NSLOT // 16], i16, tag="slot")
            nc.sync.dma_start(out=slot_sb[:, :], in_=slot_d[:, :])
            SC = S_slot // 128
            gmax = big1.tile([128, NSLOT // 128, D], f16, tag="gmax")
            for g0 in range(0, NSLOT, 1024):
                g1 = min(g0 + 1024, NSLOT)
                nc.gpsimd.dma_gather(gmax[:, g0 // 128:g1 // 128, :],
                                     h3bf[0:NSH + 128, :],
                                     slot_sb[:, (g0 // 16):(g1 // 16)],
                                     g1 - g0, g1 - g0, D,
                                     queue_num=next_q())
            mloc_f = sb.tile([128, NG], f32, tag="mlocf")
            for r in range(NG):
                red1 = sb.tile([128, D], f32, tag="red1")
                nc.vector.tensor_reduce(
                    red1[:, :],
                    gmax[:, r * SC:(r + 1) * SC, :].rearrange("p c f -> p f c"),
                    mybir.AxisListType.X, mybir.AluOpType.max)
                r1T_ps = ps.tile([128, D], f32, tag="z", name="r1T")
                nc.tensor.transpose(r1T_ps[:, :], red1[:, :], ident_sb[:, :])
                r1T_sb = sb.tile([128, D], f32, tag="r1Ts")
                nc.vector.tensor_copy(r1T_sb[:, :], r1T_ps[:, :])
                nc.vector.tensor_reduce(mloc_f[:, r:r + 1], r1T_sb[:, :],
                                        mybir.AxisListType.X,
                                        mybir.AluOpType.max)
            mlocT_full = ps.tile([128, 128], f32, tag="z")
            mlocT_ps = mlocT_full[0:NG, :]
            nc.tensor.transpose(mlocT_ps, mloc_f[:, :], ident_sb[:, :])
            mlocT_sb = sb.tile([NG, 128], f32, tag="mlocTs")
            nc.vector.tensor_copy(mlocT_sb[:, :], mlocT_ps)
            route_sb = cst.tile([NG, G], f32, tag="route")
            nc.sync.dma_start(out=route_sb[:, :], in_=route_d[:, :])
            maxT_ps = psm.tile([128, G], f32, tag="tail")
            nc.tensor.matmul(maxT_ps[:, :], mlocT_sb[:, :], route_sb[:, :],
                             start=True, stop=True)

            # ---- pool partial exchange ----
            pool_sb = sb.tile([128, 2 * G], f32, tag="poolp")
            nc.vector.tensor_copy(pool_sb[:, 0:G], meanT_ps[:, :])
            nc.vector.tensor_copy(pool_sb[:, G:2 * G], maxT_ps[:, :])
            nc.sync.dma_start(out=pool_in[:, :], in_=pool_sb[:, :])
            nc.gpsimd.collective_compute(
                "AllGather", mybir.AluOpType.bypass, replica_groups=rg,
                ins=[pool_in.opt()], outs=[pool_out.opt()])
            allp = big1.tile([128, NCORES, 2 * G], f32, tag="allp")
            nc.sync.dma_start(
                out=allp[:, :, :],
                in_=pool_out[:, :].rearrange("(c p) j -> p c j", c=NCORES))
            meanTot = sb.tile([128, G], f32, tag="meanTot")
            maxTot = sb.tile([128, G], f32, tag="maxTot")
            nc.vector.tensor_add(meanTot[:, :], allp[:, 0, 0:G],
                                 allp[:, 1, 0:G])
            nc.vector.tensor_max(maxTot[:, :], allp[:, 0, G:2 * G],
                                 allp[:, 1, G:2 * G])
            for c in range(2, NCORES):
                nc.vector.tensor_add(meanTot[:, :], meanTot[:, :],
                                     allp[:, c, 0:G])
                nc.vector.tensor_max(maxTot[:, :], maxTot[:, :],
                                     allp[:, c, G:2 * G])

            # ---- head (feature-major) ----
            W1a_sb = load_const(W1_d[0:HID, :], HID, HID, "W1a")
            W1b_sb = load_const(W1_d[HID:2 * HID, :], HID, HID, "W1b")
            W1c_sb = load_const(W1_d[2 * HID:2 * HID + G_FEAT, :], G_FEAT,
                                HID, "W1c")
            W2_sb = load_const(W2_d[:, :], HID, HID // 2, "W2")
            W3_sb = load_const(W3_d[:, :], HID // 2, 1, "W3")
            bT_sb = load_const(bT_d[:, :], HID, 3, "bT")
            gfT_sb = load_const(gfT_d[:, :], G_FEAT, G, "gfT")

            m1_ps = psm.tile([HID, G], f32, tag="tail")
            nc.tensor.matmul(m1_ps[:, :], W1a_sb[:, :], meanTot[:, :],
                             start=True, stop=False)
            nc.tensor.matmul(m1_ps[:, :], W1b_sb[:, :], maxTot[:, :],
                             start=False, stop=False)
            nc.tensor.matmul(m1_ps[:, :], W1c_sb[:, :],
                             gfT_sb[:, :], start=False, stop=True)
            m1_sb = sb.tile([HID, G], f32, tag="m1s")
            nc.scalar.activation(m1_sb[:, :], m1_ps[:, :],
                                 mybir.ActivationFunctionType.Relu,
                                 bias=bT_sb[:, 0:1])
            m2_ps = psm.tile([HID // 2, G], f32, tag="tail")
            nc.tensor.matmul(m2_ps[:, :], W2_sb[:, :], m1_sb[:, :],
                             start=True, stop=True)
            m2_sb = sb.tile([HID // 2, G], f32, tag="m2s")
            nc.scalar.activation(m2_sb[:, :], m2_ps[:, :],
                                 mybir.ActivationFunctionType.Relu,
                                 bias=bT_sb[0:HID // 2, 1:2])
            m3_ps = psm.tile([1, G], f32, tag="tail")
            nc.tensor.matmul(m3_ps[:, :], W3_sb[:, :], m2_sb[:, :],
                             start=True, stop=True)
            m3_sb = sb.tile([1, G], f32, tag="m3s")
            nc.scalar.copy(m3_sb[:, :], m3_ps[:, :])
            nc.vector.tensor_scalar_add(m3_sb[:, :], m3_sb[:, :],
                                        bT_sb[0:1, 2:3])
            nc.sync.dma_start(out=out_d[:].rearrange("(o g) -> o g", o=1),
                              in_=m3_sb[:, :])
    return nc


# ---------------- public entry ------------------------------------------------

def build_in_maps(x, edge_index, batch, g_feats, params, pre):
    x = np.asarray(x, dtype=np.float32)
    g_feats = np.asarray(g_feats, dtype=np.float32)

    bT = np.zeros((HID, 3), np.float32)
    bT[:, 0] = np.asarray(params['b1'], np.float32)
    bT[:HID // 2, 1] = np.asarray(params['b2'], np.float32)
    bT[0, 2] = np.asarray(params['b3'], np.float32).reshape(-1)[0]

    iota = np.broadcast_to(np.arange(128, dtype=np.float16), (128, 128))

    x16 = x.astype(np.float16)

    common = {
        "iota": np.ascontiguousarray(iota),
        "ident": np.eye(128, dtype=np.float32),
        "gfT": np.ascontiguousarray(g_feats.T),
        "W1": np.asarray(params['W1'], np.float32),
        "W2": np.asarray(params['W2'], np.float32),
        "W3": np.asarray(params['W3'], np.float32),
        "bT": bT,
    }
    for i in range(3):
        common[f"Wl{i}"] = np.asarray(params[f'Wl{i}'],
                                      np.float32).astype(np.float16)
        common[f"Wr{i}"] = np.asarray(params[f'Wr{i}'],
                                      np.float32).astype(np.float16)
        gb = np.zeros((HID, 2), np.float32)
        gb[:, 0] = np.asarray(params[f'gamma{i}'], np.float32)
        gb[:, 1] = np.asarray(params[f'beta{i}'], np.float32)
        common[f"gb{i}"] = gb

    in_maps = []
    for c in range(NCORES):
        xo = np.zeros((NSH_PAD, D), np.float32)
        xo[:NSH] = x[c * NSH:(c + 1) * NSH]
        sid = pre['srcids'][c]
        m0 = x16[sid].reshape(-1, CBLK, 128, D).transpose(0, 2, 1, 3)
        m = dict(common)
        m.update({
            "msgs0": np.ascontiguousarray(m0),
            "S": pre['S'][c],
            "xownT": np.ascontiguousarray(xo.T).astype(np.float16),
            "gidx": pre['gidx'][c],
            "div": pre['div'][c],
            "slot": pre['slot'][c],
            "P": pre['P'][c],
            "route": pre['route'][c],
        })
        in_maps.append(m)
    return in_maps


def build_nc(pre):
    import concourse.bacc as bacc
    nc = bacc.Bacc(None, target_bir_lowering=False, debug=False,
                   num_devices=NCORES, num_swdge_queues=4,
                   dynamic_dma_scratch_size=24576)
    nc = _build(nc, pre)
    nc.compile()
    return nc


def kernel(x, edge_index, batch, g_feats,
           Wl0, bl0, Wr0, gamma0, beta0,
           Wl1, bl1, Wr1, gamma1, beta1,
           Wl2, bl2, Wr2, gamma2, beta2,
           W1, b1, W2, b2, W3, b3):
    # bl{i} cancels inside BatchNorm (constant pre-BN shift), so it is unused.
    from concourse.bass_utils import run_bass_kernel_spmd

    params = dict(Wl0=Wl0, Wr0=Wr0, gamma0=gamma0, beta0=beta0,
                  Wl1=Wl1, Wr1=Wr1, gamma1=gamma1, beta1=beta1,
                  Wl2=Wl2, Wr2=Wr2, gamma2=gamma2, beta2=beta2,
                  W1=W1, b1=b1, W2=W2, b2=b2, W3=W3, b3=b3)
    pre = _preprocess(x, edge_index, batch)
    nc = build_nc(pre)
    in_maps = build_in_maps(x, edge_index, batch, g_feats, params, pre)
    res = run_bass_kernel_spmd(nc, in_maps, list(range(NCORES)))
    return np.asarray(res.results[0]["out"], dtype=np.float32)


# revision 21
# speedup vs baseline: 1.0027x; 1.0027x over previous
"""Distributed Bass/Trainium2 kernel for nn_AreaGNN: 3x SAGEConv(mean) +
global BatchNorm + ReLU, per-graph mean/max pooling, 3-layer MLP head.
SPMD across 8 NeuronCores; takes FULL inputs, returns FULL output [G].

v5:
- dma_gather over 4 SWDGE queues (disjoint Q7 descgen pairs, ~4x).
- One-hot S blocks generated on-chip (DVE is_equal*invdeg) from a tiny
  per-block table; first NCACHE chunks cached in SBUF across layers.
- Aggregation matmul: out[feat, dst] = msgs^T @ S (feature-major agg, no
  dense-phase transposes); agg staged f16.
- Dense phase (z matmuls + BN stat accumulation) interleaved into the chunk
  loop per-tile as each tile's aggregation completes.
- Halo exchange split into TWO region AllGathers (shard rows 0:3072 and
  3072:6272). Gather tables are laid out region-major so region-0 chunks
  start as soon as AG0 lands while AG1 (and its transposes) overlap them.
"""
import numpy as np

N = 50000
E = 800000
D = 128
HID = 128
G = 64
G_FEAT = 32
EPS = 1e-5
NCORES = 8
NSH = N // NCORES           # 6250
NSH_PAD = 6272              # 49 * 128
NTILES = NSH_PAD // 128     # 49
RSPLIT = 3072               # region split within a shard (24 tiles | 25 tiles)
K0, K1 = RSPLIT, NSH_PAD - RSPLIT          # 3072, 3200 rows/core/region
R0, R1 = K0 * NCORES, K1 * NCORES          # 24576, 25600 table rows
BLK = 128                   # edges per S block
CBLK = 32                   # blocks per gather chunk (4096 edges)
CH = BLK * CBLK
CPW = CH // 16
NCACHE = 4                  # S chunks cached in SBUF across layers
TABLE_SHARED = True


# ---------------- host-side preprocessing -----------------------------------

def _wrap_idx(idx, ch):
    """[L] -> [L/ch, 128, ch/16] int16: element m of a chunk at (m%16, m//16),
    replicated across the eight 16-partition groups."""
    L = idx.shape[0]
    out = np.empty((L // ch, 128, ch // 16), dtype=np.int16)
    w = idx.reshape(L // ch, ch // 16, 16).transpose(0, 2, 1)
    for g in range(8):
        out[:, g * 16:(g + 1) * 16, :] = w
    return out


def _preprocess(x, edge_index, batch):
    src = np.asarray(edge_index[0], dtype=np.int64)
    dst = np.asarray(edge_index[1], dtype=np.int64)
    batch = np.asarray(batch, dtype=np.int64)

    indeg = np.bincount(dst, minlength=N)
    invdeg_all = (1.0 / np.maximum(indeg, 1.0)).astype(np.float32)

    core_of = dst // NSH
    tile_of = (dst % NSH) // 128
    # region of src within its owner shard + region-relative table index
    src_core = src // NSH
    src_off = src % NSH
    half_of = (src_off >= RSPLIT).astype(np.int64)
    src_reg_idx = np.where(half_of == 0,
                           src_core * K0 + src_off,
                           src_core * K1 + (src_off - RSPLIT))

    counts = np.zeros((NCORES, 2, NTILES), dtype=np.int64)
    buckets = {}
    for c in range(NCORES):
        mc = core_of == c
        for h in range(2):
            mh = mc & (half_of == h)
            for t in range(NTILES):
                m = mh & (tile_of == t)
                g = src_reg_idx[m]
                d = (dst[m] % NSH) % 128        # dst within tile
                w = invdeg_all[dst[m]]
                buckets[(c, h, t)] = (g, d, w)
                counts[c, h, t] = len(g)

    # global block schedule: both halves get >= 1 block per tile (pass-A copy
    # initializes agg; pass-B stop triggers the interleaved dense step)
    nblk = np.ceil(counts.max(axis=0) / BLK).astype(np.int64)  # [2, NTILES]
    nblk = np.maximum(nblk, 1)
    extra = [0, 0]
    for h in range(2):
        tot = int(nblk[h].sum())
        extra[h] = (-tot) % CBLK
    sched = []   # list of (h, t) per block, in execution order
    for h in range(2):
        for t in range(NTILES):
            sched += [(h, t)] * int(nblk[h, t])
        sched += [(h, NTILES - 1)] * extra[h]
    nblk_tot = len(sched)
    nchunks = nblk_tot // CBLK
    assert nchunks * CBLK == nblk_tot
    chunk_half = [sched[k * CBLK][0] for k in range(nchunks)]
    for k in range(nchunks):
        assert all(sched[k * CBLK + j][0] == chunk_half[k] for j in range(CBLK))

    # per-chunk gather pieces [(col offset in chunk, num_idxs)]
    run_start = {}
    b0 = 0
    for h in range(2):
        for t in range(NTILES):
            nb = int(nblk[h, t]) + (extra[h] if t == NTILES - 1 else 0)
            run_start[(h, t)] = (b0, nb)
            b0 += nb
    r16 = {k: min(-(-int(counts[:, k[0], k[1]].max()) // 16) * 16,
                  run_start[k][1] * BLK)
           for k in run_start}
    gather_pieces = []
    for k in range(nchunks):
        c0, c1 = k * CBLK * BLK, (k + 1) * CBLK * BLK
        iv = []
        for (h, t), (rb, nb) in run_start.items():
            if h != chunk_half[k]:
                continue
            s0, s1 = rb * BLK, rb * BLK + r16[(h, t)]
            a, b = max(s0, c0), min(s1, c1)
            if a < b:
                iv.append((a - c0, b - c0))
        pieces = [(q, 1024) for q in range(0, CBLK * BLK, 1024)]
        gather_pieces.append(pieces)

    # per-core gather idx + per-block [dstidx, invdeg] following the schedule
    # (also original src node ids per slot, for the layer-0 host pregather)
    gidx_cores, div_cores, srcids_cores, S_cores = [], [], [], []
    src_orig = {}
    for c in range(NCORES):
        mc = core_of == c
        for h in range(2):
            mh = mc & (half_of == h)
            for t in range(NTILES):
                m = mh & (tile_of == t)
                src_orig[(c, h, t)] = src[m]
    for c in range(NCORES):
        gi = np.zeros(nblk_tot * BLK, dtype=np.int64)
        sid = np.zeros(nblk_tot * BLK, dtype=np.int64)
        div = np.zeros((nblk_tot * BLK, 2), dtype=np.float32)
        b0 = 0
        for h in range(2):
            for t in range(NTILES):
                nb = int(nblk[h, t]) + (extra[h] if t == NTILES - 1 else 0)
                g, d, w = buckets[(c, h, t)]
                n = len(g)
                gi[b0 * BLK: b0 * BLK + n] = g
                sid[b0 * BLK: b0 * BLK + n] = src_orig[(c, h, t)]
                div[b0 * BLK: b0 * BLK + n, 0] = d.astype(np.float32)
                div[b0 * BLK: b0 * BLK + n, 1] = w.astype(np.float32)
                b0 += nb
        assert b0 == nblk_tot
        gidx_cores.append(_wrap_idx(gi.astype(np.int16), CH))
        srcids_cores.append(sid)
        div_cores.append(np.ascontiguousarray(
            div.reshape(nblk_tot, BLK, 2).transpose(1, 0, 2)))
        Sm = np.zeros((nblk_tot * BLK, 128), dtype=np.float16)
        slots = np.arange(nblk_tot * BLK)
        Sm[slots, div[:, 0].astype(np.int64)] = div[:, 1].astype(np.float16)
        S_cores.append(np.ascontiguousarray(
            Sm.reshape(nblk_tot, BLK, 128).transpose(1, 0, 2)
            .reshape(BLK, nblk_tot * 128)))

    # last pass-B block index per tile (the interleaved dense trigger)
    pbstop = {}
    for b, (h, t) in enumerate(sched):
        if h == 1:
            pbstop[t] = b
    dense_after = {b: t for t, b in pbstop.items()}

    cnt_g = np.bincount(batch, minlength=G)
    inv_cnt = (1.0 / np.maximum(cnt_g, 1.0)).astype(np.float32)

    P = []
    for c in range(NCORES):
        p = np.zeros((NSH_PAD, G), dtype=np.float32)
        b = batch[c * NSH:(c + 1) * NSH]
        p[np.arange(NSH), b] = inv_cnt[b]
        P.append(p)

    NG, Smax = 0, 0
    groups_c = []
    for c in range(NCORES):
        b = batch[c * NSH:(c + 1) * NSH]
        glo, ghi = int(b.min()), int(b.max())
        groups = [(g, np.where(b == g)[0]) for g in range(glo, ghi + 1)]
        groups_c.append((glo, groups))
        NG = max(NG, ghi - glo + 1)
        Smax = max(Smax, max(len(gr) for _, gr in groups))
    S_slot = ((Smax + 127) // 128) * 128
    slot, route = [], []
    for c in range(NCORES):
        glo, groups = groups_c[c]
        sm = np.full(NG * S_slot, NSH, dtype=np.int64)   # NSH = zero dummy row
        R = np.zeros((NG, G), dtype=np.float32)
        for g, gr in groups:
            r = g - glo
            sm[r * S_slot:r * S_slot + len(gr)] = gr
            R[r, g] = 1.0
        slot.append(_wrap_idx(sm.astype(np.int16), NG * S_slot)[0])
        route.append(R)

    return dict(nblk=nblk, extra=extra, sched=sched, nblk_tot=nblk_tot,
                nchunks=nchunks, chunk_half=chunk_half,
                gather_pieces=gather_pieces, dense_after=dense_after,
                gidx=gidx_cores, div=div_cores, srcids=srcids_cores,
                S=S_cores, P=P,
                slot=slot, route=route, S_slot=S_slot, NG=NG)


# ---------------- device kernel builder --------------------------------------

def _build(nc, pre):
    import concourse.mybir as mybir
    import concourse.tile as tile

    f32 = mybir.dt.float32
    f16 = mybir.dt.float16
    i16 = mybir.dt.int16
    NCH = pre['nchunks']
    NBLK_TOT = pre['nblk_tot']
    NG, S_slot = pre['NG'], pre['S_slot']
    NSLOT = NG * S_slot
    sched = pre['sched']
    dense_after = pre['dense_after']

    is_start = [True] * NBLK_TOT
    is_stop = [True] * NBLK_TOT
    for b in range(NBLK_TOT):
        if b > 0 and sched[b] == sched[b - 1]:
            is_start[b] = False
        if b < NBLK_TOT - 1 and sched[b] == sched[b + 1]:
            is_stop[b] = False
    # block position within its run + run length (for A/B psum splitting)
    run_pos = [0] * NBLK_TOT
    run_len = [0] * NBLK_TOT
    p = 0
    for b in range(NBLK_TOT):
        run_pos[b] = p
        p = 0 if is_stop[b] else p + 1
    L = 0
    for b in range(NBLK_TOT - 1, -1, -1):
        if is_stop[b]:
            L = run_pos[b] + 1
        run_len[b] = L

    qctr = [0]

    def next_q():
        q = 1 + qctr[0] % 3
        qctr[0] += 1
        return q

    # ---- I/O ----
    msgs0_d = nc.dram_tensor("msgs0", [NCH, 128, CBLK, D], f16,
                             kind="ExternalInput")
    S_d = nc.dram_tensor("S", [BLK, NBLK_TOT * 128], f16,
                         kind="ExternalInput")
    xownT = nc.dram_tensor("xownT", [128, NSH_PAD], f16, kind="ExternalInput")
    gidx_d = nc.dram_tensor("gidx", [NCH, 128, CPW], i16, kind="ExternalInput")
    div_d = nc.dram_tensor("div", [128, NBLK_TOT, 2], f32,
                           kind="ExternalInput")
    iota_d = nc.dram_tensor("iota", [128, 128], f16, kind="ExternalInput")
    slot_d = nc.dram_tensor("slot", [128, NSLOT // 16], i16, kind="ExternalInput")
    P_d = nc.dram_tensor("P", [NSH_PAD, G], f32, kind="ExternalInput")
    route_d = nc.dram_tensor("route", [NG, G], f32, kind="ExternalInput")
    gfT_d = nc.dram_tensor("gfT", [G_FEAT, G], f32, kind="ExternalInput")
    ident_d = nc.dram_tensor("ident", [128, 128], f32, kind="ExternalInput")
    Wl_d = [nc.dram_tensor(f"Wl{i}", [D, HID], f16, kind="ExternalInput")
            for i in range(3)]
    Wr_d = [nc.dram_tensor(f"Wr{i}", [D, HID], f16, kind="ExternalInput")
            for i in range(3)]
    gb_d = [nc.dram_tensor(f"gb{i}", [HID, 2], f32, kind="ExternalInput")
            for i in range(3)]
    W1_d = nc.dram_tensor("W1", [2 * HID + G_FEAT, HID], f32, kind="ExternalInput")
    W2_d = nc.dram_tensor("W2", [HID, HID // 2], f32, kind="ExternalInput")
    W3_d = nc.dram_tensor("W3", [HID // 2, 1], f32, kind="ExternalInput")
    bT_d = nc.dram_tensor("bT", [HID, 3], f32, kind="ExternalInput")

    out_d = nc.dram_tensor("out", [G], f32, kind="ExternalOutput")

    rg = [list(range(NCORES))]

    with tile.TileContext(nc) as tc:
        with (
            tc.tile_pool(name="sb", bufs=3) as sb,
            tc.tile_pool(name="big", bufs=2) as bigp,       # zT f16 ring
            tc.tile_pool(name="agg", bufs=1) as aggp,       # agg f16
            tc.tile_pool(name="big1", bufs=1) as big1,      # gmax/allp
            tc.tile_pool(name="msg", bufs=4) as msgp,
            tc.tile_pool(name="scache", bufs=1) as scp,
            tc.tile_pool(name="idx", bufs=4) as idxp,
            tc.tile_pool(name="cst", bufs=1) as cst,
            tc.tile_pool(name="ps", bufs=1, space="PSUM") as ps,
            tc.tile_pool(name="pst", bufs=2, space="PSUM") as pst,
            tc.tile_pool(name="psa", bufs=3, space="PSUM") as psa,
            tc.tile_pool(name="psm", bufs=1, space="PSUM") as psm,
            tc.tile_pool(name="dram", bufs=1, space="DRAM") as dram,
        ):
            # ---- DRAM scratch ----
            hbounce = [dram.tile([NSH_PAD, D], f16, tag=f"hb{i}", name=f"hb{i}")
                       for i in range(2)]
            tblA = [dram.tile([R0, D], f16, tag=f"tblA{i}", name=f"tblA{i}",
                              addr_space="Shared" if TABLE_SHARED else "Local")
                    for i in range(2)]
            tblB = [dram.tile([R1, D], f16, tag=f"tblB{i}", name=f"tblB{i}",
                              addr_space="Shared" if TABLE_SHARED else "Local")
                    for i in range(2)]
            h3bf = dram.tile([NSH + 128, D], f16, tag="h3bf")
            stats_in = [dram.tile([D, 2], f32, tag=f"stats_in{i}",
                                  name=f"stats_in{i}") for i in range(3)]
            stats_out = [dram.tile([NCORES * D, 2], f32, tag=f"stats_out{i}",
                                   name=f"stats_out{i}", addr_space="Shared")
                         for i in range(3)]
            pool_in = dram.tile([D, 2 * G], f32, tag="pool_in")
            pool_out = dram.tile([NCORES * D, 2 * G], f32, tag="pool_out",
                                 addr_space="Shared")

            def load_const(src_ap, rows, cols, name, dt=f32):
                t = cst.tile([rows, cols], dt, tag=name)
                nc.sync.dma_start(out=t[:, :], in_=src_ap)
                return t

            ident_sb = load_const(ident_d[:, :], 128, 128, "ident")
            ident16_sb = cst.tile([128, 128], f16, tag="ident16")
            nc.vector.tensor_copy(ident16_sb[:, :], ident_sb[:, :])
            iota_sb = load_const(iota_d[:, :], 128, 128, "iota", f16)
            div_sb = cst.tile([128, NBLK_TOT, 2], f32, tag="div")
            nc.sync.dma_start(out=div_sb[:, :, :], in_=div_d[:, :, :])
            xT_sb = cst.tile([128, NSH_PAD], f16, tag="xT")
            nc.sync.dma_start(out=xT_sb[:, :], in_=xownT[:, :])

            scache = [scp.tile([128, CBLK, 128], f16, tag=f"Sc{k}",
                               name=f"Sc{k}")
                      for k in range(NCACHE)]

            hT_prev = xT_sb

            for li in range(3):
                Wl_sb = load_const(Wl_d[li][:, :], D, HID, f"Wl{li}", f16)
                Wr_sb = load_const(Wr_d[li][:, :], D, HID, f"Wr{li}", f16)
                gb_sb = load_const(gb_d[li][:, :], HID, 2, f"gb{li}")

                agg_sb = aggp.tile([128, NSH_PAD], f16, tag="agg")
                zT = bigp.tile([128, NSH_PAD], f16, tag="zT")
                zsum = sb.tile([128, NTILES], f32, tag="zsum")
                zsq = sb.tile([128, NTILES], f32, tag="zsq")

                # ---- chunk loop: gather + S + agg matmuls + inline dense ----
                acc_ps = None
                for k in range(NCH):
                    h = pre['chunk_half'][k]
                    mtiles = [msgp.tile([128, 8, D], f16, tag=f"msgs{p}",
                                        name=f"m{p}")
                              for p in range(4)]
                    if li == 0:
                        for p in range(4):
                            nc.sync.dma_start(
                                out=mtiles[p][:, :, :],
                                in_=msgs0_d[k, :, p * 8:(p + 1) * 8, :])
                    else:
                        src_tab = (tblA[(li - 1) % 2][:, :] if h == 0
                                   else tblB[(li - 1) % 2][:, :])
                        gi = idxp.tile([128, CPW], i16, tag="gi")
                        nc.sync.dma_start(out=gi[:], in_=gidx_d[k, :, :])
                        for off, n in pre['gather_pieces'][k]:
                            p = off // 1024
                            nc.gpsimd.dma_gather(
                                mtiles[p][:, 0:(n + 127) // 128, :], src_tab,
                                gi[:, off // 16:off // 16 + n // 16], n, n, D,
                                queue_num=next_q())
                    if k < NCACHE:
                        S_sb = scache[k]
                    else:
                        S_sb = msgp.tile([128, CBLK, 128], f16, tag="Ssb")
                    if li == 0:
                        nc.sync.dma_start(
                            out=S_sb[:, :, :],
                            in_=S_d[:, k * CBLK * 128:(k + 1) * CBLK * 128]
                            .rearrange("p (j d) -> p j d", d=128))
                    for j in range(CBLK):
                        b = k * CBLK + j
                        h_b, t_b = sched[b]
                        if li > 0 and k >= NCACHE:
                            # S[slot, dstcol] = (iota == dstidx[slot])*invdeg
                            nc.vector.tensor_scalar(
                                S_sb[:, j, :], iota_sb[:, :],
                                div_sb[:, b, 0:1], div_sb[:, b, 1:2],
                                mybir.AluOpType.is_equal,
                                mybir.AluOpType.mult)
                        if is_start[b]:
                            acc_ps = psa.tile([128, D], f32, tag="accp")
                        nc.tensor.matmul(acc_ps[:, :],
                                         mtiles[j // 8][:, j % 8, :],
                                         S_sb[:, j, :],
                                         start=is_start[b], stop=is_stop[b])
                        if is_stop[b]:
                            sl = agg_sb[:, t_b * 128:(t_b + 1) * 128]
                            if h_b == 0:
                                nc.scalar.copy(sl, acc_ps[:, :])
                            else:
                                nc.vector.tensor_add(sl, sl, acc_ps[:, :])
                            t_d = dense_after.get(b)
                            if t_d is not None:
                                z_ps = ps.tile([128, D], f32, tag="z")
                                nc.tensor.matmul(
                                    z_ps[:, :], Wl_sb[:, :],
                                    agg_sb[:, t_d * 128:(t_d + 1) * 128],
                                    start=True, stop=False)
                                nc.tensor.matmul(
                                    z_ps[:, :], Wr_sb[:, :],
                                    hT_prev[:, t_d * 128:(t_d + 1) * 128],
                                    start=False, stop=True)
                                nc.scalar.activation(
                                    zT[:, t_d * 128:(t_d + 1) * 128],
                                    z_ps[:, :],
                                    mybir.ActivationFunctionType.Copy,
                                    accum_out=zsum[:, t_d:t_d + 1])
                                sq_scr = sb.tile([128, D], f32, tag="sqscr")
                                nc.scalar.activation(
                                    sq_scr[:, :],
                                    zT[:, t_d * 128:(t_d + 1) * 128],
                                    mybir.ActivationFunctionType.Square,
                                    accum_out=zsq[:, t_d:t_d + 1])

                # ---- BN stats exchange + scale/shift ----
                stat_sb = sb.tile([128, 2], f32, tag="stat")
                nc.vector.tensor_reduce(stat_sb[:, 0:1], zsum[:, :],
                                        mybir.AxisListType.X,
                                        mybir.AluOpType.add)
                nc.vector.tensor_reduce(stat_sb[:, 1:2], zsq[:, :],
                                        mybir.AxisListType.X,
                                        mybir.AluOpType.add)
                nc.sync.dma_start(out=stats_in[li][:, :], in_=stat_sb[:, :])
                nc.gpsimd.collective_compute(
                    "AllGather", mybir.AluOpType.bypass, replica_groups=rg,
                    ins=[stats_in[li].opt()], outs=[stats_out[li].opt()])
                allst = sb.tile([128, NCORES, 2], f32, tag="allst")
                nc.sync.dma_start(
                    out=allst[:, :, :],
                    in_=stats_out[li][:, :].rearrange("(c p) j -> p c j",
                                                      c=NCORES))
                tot = sb.tile([128, 2], f32, tag="tot")
                nc.vector.tensor_add(tot[:, :], allst[:, 0, :], allst[:, 1, :])
                for c in range(2, NCORES):
                    nc.vector.tensor_add(tot[:, :], tot[:, :], allst[:, c, :])
                mu = sb.tile([128, 6], f32, tag="mu")
                nc.scalar.mul(mu[:, 0:1], tot[:, 0:1], 1.0 / N)
                nc.scalar.mul(mu[:, 1:2], tot[:, 1:2], 1.0 / N)
                nc.vector.tensor_mul(mu[:, 2:3], mu[:, 0:1], mu[:, 0:1])
                nc.vector.tensor_sub(mu[:, 3:4], mu[:, 1:2], mu[:, 2:3])
                nc.vector.tensor_scalar_add(mu[:, 3:4], mu[:, 3:4], EPS)
                nc.vector.reciprocal(mu[:, 4:5], mu[:, 3:4])
                nc.scalar.sqrt(mu[:, 4:5], mu[:, 4:5])
                nc.vector.tensor_mul(mu[:, 4:5], mu[:, 4:5], gb_sb[:, 0:1])
                nc.vector.tensor_mul(mu[:, 5:6], mu[:, 0:1], mu[:, 4:5])
                nc.vector.tensor_sub(mu[:, 5:6], gb_sb[:, 1:2], mu[:, 5:6])

                # ---- relu (real cols; pads stay 0) + per-tile transpose/ship
                if li < 2:
                    hb = hbounce[li % 2]
                    for ck in range(13):
                        w = 512 if ck < 12 else NSH - 12 * 512
                        nc.scalar.activation(zT[:, ck * 512:ck * 512 + w],
                                             zT[:, ck * 512:ck * 512 + w],
                                             mybir.ActivationFunctionType.Relu,
                                             bias=mu[:, 5:6], scale=mu[:, 4:5])
                        t0, t1 = ck * 4, min(ck * 4 + 4, NTILES)
                        for t in range(t0, t1):
                            hT_ps = pst.tile([128, D], f16, tag="tp16")
                            nc.tensor.transpose(
                                hT_ps[:, :], zT[:, t * 128:(t + 1) * 128],
                                ident16_sb[:, :])
                            hbf_sb = sb.tile([128, D], f16, tag="hbf")
                            nc.vector.tensor_copy(hbf_sb[:, :], hT_ps[:, :])
                            nc.sync.dma_start(
                                out=hb[t * 128:(t + 1) * 128, :],
                                in_=hbf_sb[:, :])
                        if t1 == 24:   # region 0 shipped -> AG0
                            nc.gpsimd.collective_compute(
                                "AllGather", mybir.AluOpType.bypass,
                                replica_groups=rg,
                                ins=[hb[0:RSPLIT, :].opt()],
                                outs=[tblA[li % 2].opt()])
                    nc.gpsimd.collective_compute(
                        "AllGather", mybir.AluOpType.bypass, replica_groups=rg,
                        ins=[hb[RSPLIT:NSH_PAD, :].opt()],
                        outs=[tblB[li % 2].opt()])
                else:
                    meanT_ps = psm.tile([128, G], f32, tag="meanT")
                    for ck in range(13):
                        w = 512 if ck < 12 else NSH - 12 * 512
                        nc.scalar.activation(zT[:, ck * 512:ck * 512 + w],
                                             zT[:, ck * 512:ck * 512 + w],
                                             mybir.ActivationFunctionType.Relu,
                                             bias=mu[:, 5:6], scale=mu[:, 4:5])
                        t0, t1 = ck * 4, min(ck * 4 + 4, NTILES)
                        for t in range(t0, t1):
                            hT_ps = pst.tile([128, D], f16, tag="tp16")
                            nc.tensor.transpose(
                                hT_ps[:, :], zT[:, t * 128:(t + 1) * 128],
                                ident16_sb[:, :])
                            h3_sb = sb.tile([128, D], f32, tag="h3")
                            nc.vector.tensor_copy(h3_sb[:, :], hT_ps[:, :])
                            P_sb = sb.tile([128, G], f32, tag="P")
                            nc.sync.dma_start(
                                out=P_sb[:, :],
                                in_=P_d[t * 128:(t + 1) * 128, :])
                            nc.tensor.matmul(meanT_ps[:, :], h3_sb[:, :],
                                             P_sb[:, :],
                                             start=(t == 0),
                                             stop=(t == NTILES - 1))
                            hbf_sb = sb.tile([128, D], f16, tag="hbf")
                            nc.vector.tensor_copy(hbf_sb[:, :], h3_sb[:, :])
                            nc.sync.dma_start(
                                out=h3bf[t * 128:(t + 1) * 128, :],
                                in_=hbf_sb[:, :])
                    zrow = sb.tile([1, D], f16, tag="zrow")
                    nc.vector.memset(zrow[:, :], 0.0)
                    nc.sync.dma_start(out=h3bf[NSH:NSH + 1, :], in_=zrow[:, :])
                hT_prev = zT

            # ---- max pool: transpose-gather + segmented max + route ----
            slot_sb = cst.tile([128, NSLOT // 16], i16, tag="slot")
            nc.sync.dma_start(out=slot_sb[:, :], in_=slot_d[:, :])
            SC = S_slot // 128
            gmax = big1.tile([128, NSLOT // 128, D], f16, tag="gmax")
            for g0 in range(0, NSLOT, 1024):
                g1 = min(g0 + 1024, NSLOT)
                nc.gpsimd.dma_gather(gmax[:, g0 // 128:g1 // 128, :],
                                     h3bf[0:NSH + 128, :],
                                     slot_sb[:, (g0 // 16):(g1 // 16)],
                                     g1 - g0, g1 - g0, D,
                                     queue_num=next_q())
            mloc_f = sb.tile([128, NG], f32, tag="mlocf")
            for r in range(NG):
                red1 = sb.tile([128, D], f32, tag="red1")
                nc.vector.tensor_reduce(
                    red1[:, :],
                    gmax[:, r * SC:(r + 1) * SC, :].rearrange("p c f -> p f c"),
                    mybir.AxisListType.X, mybir.AluOpType.max)
                r1T_ps = ps.tile([128, D], f32, tag="z", name="r1T")
                nc.tensor.transpose(r1T_ps[:, :], red1[:, :], ident_sb[:, :])
                r1T_sb = sb.tile([128, D], f32, tag="r1Ts")
                nc.vector.tensor_copy(r1T_sb[:, :], r1T_ps[:, :])
                nc.vector.tensor_reduce(mloc_f[:, r:r + 1], r1T_sb[:, :],
                                        mybir.AxisListType.X,
                                        mybir.AluOpType.max)
            mlocT_full = ps.tile([128, 128], f32, tag="z")
            mlocT_ps = mlocT_full[0:NG, :]
            nc.tensor.transpose(mlocT_ps, mloc_f[:, :], ident_sb[:, :])
            mlocT_sb = sb.tile([NG, 128], f32, tag="mlocTs")
            nc.vector.tensor_copy(mlocT_sb[:, :], mlocT_ps)
            route_sb = cst.tile([NG, G], f32, tag="route")
            nc.sync.dma_start(out=route_sb[:, :], in_=route_d[:, :])
            maxT_ps = psm.tile([128, G], f32, tag="tail")
            nc.tensor.matmul(maxT_ps[:, :], mlocT_sb[:, :], route_sb[:, :],
                             start=True, stop=True)

            # ---- pool partial exchange ----
            pool_sb = sb.tile([128, 2 * G], f32, tag="poolp")
            nc.vector.tensor_copy(pool_sb[:, 0:G], meanT_ps[:, :])
            nc.vector.tensor_copy(pool_sb[:, G:2 * G], maxT_ps[:, :])
            nc.sync.dma_start(out=pool_in[:, :], in_=pool_sb[:, :])
            nc.gpsimd.collective_compute(
                "AllGather", mybir.AluOpType.bypass, replica_groups=rg,
                ins=[pool_in.opt()], outs=[pool_out.opt()])
            allp = big1.tile([128, NCORES, 2 * G], f32, tag="allp")
            nc.sync.dma_start(
                out=allp[:, :, :],
                in_=pool_out[:, :].rearrange("(c p) j -> p c j", c=NCORES))
            meanTot = sb.tile([128, G], f32, tag="meanTot")
            maxTot = sb.tile([128, G], f32, tag="maxTot")
            nc.vector.tensor_add(meanTot[:, :], allp[:, 0, 0:G],
                                 allp[:, 1, 0:G])
            nc.vector.tensor_max(maxTot[:, :], allp[:, 0, G:2 * G],
                                 allp[:, 1, G:2 * G])
            for c in range(2, NCORES):
                nc.vector.tensor_add(meanTot[:, :], meanTot[:, :],
                                     allp[:, c, 0:G])
                nc.vector.tensor_max(maxTot[:, :], maxTot[:, :],
                                     allp[:, c, G:2 * G])

            # ---- head (feature-major) ----
            W1a_sb = load_const(W1_d[0:HID, :], HID, HID, "W1a")
            W1b_sb = load_const(W1_d[HID:2 * HID, :], HID, HID, "W1b")
            W1c_sb = load_const(W1_d[2 * HID:2 * HID + G_FEAT, :], G_FEAT,
                                HID, "W1c")
            W2_sb = load_const(W2_d[:, :], HID, HID // 2, "W2")
            W3_sb = load_const(W3_d[:, :], HID // 2, 1, "W3")
            bT_sb = load_const(bT_d[:, :], HID, 3, "bT")
            gfT_sb = load_const(gfT_d[:, :], G_FEAT, G, "gfT")

            m1_ps = psm.tile([HID, G], f32, tag="tail")
            nc.tensor.matmul(m1_ps[:, :], W1a_sb[:, :], meanTot[:, :],
                             start=True, stop=False)
            nc.tensor.matmul(m1_ps[:, :], W1b_sb[:, :], maxTot[:, :],
                             start=False, stop=False)
            nc.tensor.matmul(m1_ps[:, :], W1c_sb[:, :],
                             gfT_sb[:, :], start=False, stop=True)
            m1_sb = sb.tile([HID, G], f32, tag="m1s")
            nc.scalar.activation(m1_sb[:, :], m1_ps[:, :],
                                 mybir.ActivationFunctionType.Relu,
                                 bias=bT_sb[:, 0:1])
            m2_ps = psm.tile([HID // 2, G], f32, tag="tail")
            nc.tensor.matmul(m2_ps[:, :], W2_sb[:, :], m1_sb[:, :],
                             start=True, stop=True)
            m2_sb = sb.tile([HID // 2, G], f32, tag="m2s")
            nc.scalar.activation(m2_sb[:, :], m2_ps[:, :],
                                 mybir.ActivationFunctionType.Relu,
                                 bias=bT_sb[0:HID // 2, 1:2])
            m3_ps = psm.tile([1, G], f32, tag="tail")
            nc.tensor.matmul(m3_ps[:, :], W3_sb[:, :], m2_sb[:, :],
                             start=True, stop=True)
            m3_sb = sb.tile([1, G], f32, tag="m3s")
            nc.scalar.copy(m3_sb[:, :], m3_ps[:, :])
            nc.vector.tensor_scalar_add(m3_sb[:, :], m3_sb[:, :],
                                        bT_sb[0:1, 2:3])
            nc.sync.dma_start(out=out_d[:].rearrange("(o g) -> o g", o=1),
                              in_=m3_sb[:, :])
    return nc


# ---------------- public entry ------------------------------------------------

def build_in_maps(x, edge_index, batch, g_feats, params, pre):
    x = np.asarray(x, dtype=np.float32)
    g_feats = np.asarray(g_feats, dtype=np.float32)

    bT = np.zeros((HID, 3), np.float32)
    bT[:, 0] = np.asarray(params['b1'], np.float32)
    bT[:HID // 2, 1] = np.asarray(params['b2'], np.float32)
    bT[0, 2] = np.asarray(params['b3'], np.float32).reshape(-1)[0]

    iota = np.broadcast_to(np.arange(128, dtype=np.float16), (128, 128))

    x16 = x.astype(np.float16)

    common = {
        "iota": np.ascontiguousarray(iota),
        "ident": np.eye(128, dtype=np.float32),
        "gfT": np.ascontiguousarray(g_feats.T),
        "W1": np.asarray(params['W1'], np.float32),
        "W2": np.asarray(params['W2'], np.float32),
        "W3": np.asarray(params['W3'], np.float32),
        "bT": bT,
    }
    for i in range(3):
        common[f"Wl{i}"] = np.asarray(params[f'Wl{i}'],
                                      np.float32).astype(np.float16)
        common[f"Wr{i}"] = np.asarray(params[f'Wr{i}'],
                                      np.float32).astype(np.float16)
        gb = np.zeros((HID, 2), np.float32)
        gb[:, 0] = np.asarray(params[f'gamma{i}'], np.float32)
        gb[:, 1] = np.asarray(params[f'beta{i}'], np.float32)
        common[f"gb{i}"] = gb

    in_maps = []
    for c in range(NCORES):
        xo = np.zeros((NSH_PAD, D), np.float32)
        xo[:NSH] = x[c * NSH:(c + 1) * NSH]
        sid = pre['srcids'][c]
        m0 = x16[sid].reshape(-1, CBLK, 128, D).transpose(0, 2, 1, 3)
        m = dict(common)
        m.update({
            "msgs0": np.ascontiguousarray(m0),
            "S": pre['S'][c],
            "xownT": np.ascontiguousarray(xo.T).astype(np.float16),
            "gidx": pre['gidx'][c],
            "div": pre['div'][c],
            "slot": pre['slot'][c],
            "P": pre['P'][c],
            "route": pre['route'][c],
        })
        in_maps.append(m)
    return in_maps


def build_nc(pre):
    import concourse.bacc as bacc
    nc = bacc.Bacc(None, target_bir_lowering=False, debug=False,
                   num_devices=NCORES, num_swdge_queues=4,
                   dynamic_dma_scratch_size=24576)
    nc = _build(nc, pre)
    nc.compile()
    return nc


def kernel(x, edge_index, batch, g_feats,
           Wl0, bl0, Wr0, gamma0, beta0,
           Wl1, bl1, Wr1, gamma1, beta1,
           Wl2, bl2, Wr2, gamma2, beta2,
           W1, b1, W2, b2, W3, b3):
    # bl{i} cancels inside BatchNorm (constant pre-BN shift), so it is unused.
    from concourse.bass_utils import run_bass_kernel_spmd

    params = dict(Wl0=Wl0, Wr0=Wr0, gamma0=gamma0, beta0=beta0,
                  Wl1=Wl1, Wr1=Wr1, gamma1=gamma1, beta1=beta1,
                  Wl2=Wl2, Wr2=Wr2, gamma2=gamma2, beta2=beta2,
                  W1=W1, b1=b1, W2=W2, b2=b2, W3=W3, b3=b3)
    pre = _preprocess(x, edge_index, batch)
    nc = build_nc(pre)
    in_maps = build_in_maps(x, edge_index, batch, g_feats, params, pre)
    res = run_bass_kernel_spmd(nc, in_maps, list(range(NCORES)))
    return np.asarray(res.results[0]["out"], dtype=np.float32)


# revision 22
# speedup vs baseline: 1.0317x; 1.0288x over previous
"""Distributed Bass/Trainium2 kernel for nn_AreaGNN: 3x SAGEConv(mean) +
global BatchNorm + ReLU, per-graph mean/max pooling, 3-layer MLP head.
SPMD across 8 NeuronCores; takes FULL inputs, returns FULL output [G].

v5:
- dma_gather over 4 SWDGE queues (disjoint Q7 descgen pairs, ~4x).
- One-hot S blocks generated on-chip (DVE is_equal*invdeg) from a tiny
  per-block table; first NCACHE chunks cached in SBUF across layers.
- Aggregation matmul: out[feat, dst] = msgs^T @ S (feature-major agg, no
  dense-phase transposes); agg staged f16.
- Dense phase (z matmuls + BN stat accumulation) interleaved into the chunk
  loop per-tile as each tile's aggregation completes.
- Halo exchange split into TWO region AllGathers (shard rows 0:3072 and
  3072:6272). Gather tables are laid out region-major so region-0 chunks
  start as soon as AG0 lands while AG1 (and its transposes) overlap them.
"""
import numpy as np

N = 50000
E = 800000
D = 128
HID = 128
G = 64
G_FEAT = 32
EPS = 1e-5
NCORES = 8
NSH = N // NCORES           # 6250
NSH_PAD = 6272              # 49 * 128
NTILES = NSH_PAD // 128     # 49
RSPLIT = 3072               # region split within a shard (24 tiles | 25 tiles)
K0, K1 = RSPLIT, NSH_PAD - RSPLIT          # 3072, 3200 rows/core/region
R0, R1 = K0 * NCORES, K1 * NCORES          # 24576, 25600 table rows
BLK = 128                   # edges per S block
CBLK = 32                   # blocks per gather chunk (4096 edges)
CH = BLK * CBLK
CPW = CH // 16
NCACHE = 4                  # S chunks cached in SBUF across layers
TABLE_SHARED = True


# ---------------- host-side preprocessing -----------------------------------

def _wrap_idx(idx, ch):
    """[L] -> [L/ch, 128, ch/16] int16: element m of a chunk at (m%16, m//16),
    replicated across the eight 16-partition groups."""
    L = idx.shape[0]
    out = np.empty((L // ch, 128, ch // 16), dtype=np.int16)
    w = idx.reshape(L // ch, ch // 16, 16).transpose(0, 2, 1)
    for g in range(8):
        out[:, g * 16:(g + 1) * 16, :] = w
    return out


def _preprocess(x, edge_index, batch):
    src = np.asarray(edge_index[0], dtype=np.int64)
    dst = np.asarray(edge_index[1], dtype=np.int64)
    batch = np.asarray(batch, dtype=np.int64)

    indeg = np.bincount(dst, minlength=N)
    invdeg_all = (1.0 / np.maximum(indeg, 1.0)).astype(np.float32)

    core_of = dst // NSH
    tile_of = (dst % NSH) // 128
    # region of src within its owner shard + region-relative table index
    src_core = src // NSH
    src_off = src % NSH
    half_of = (src_off >= RSPLIT).astype(np.int64)
    src_reg_idx = np.where(half_of == 0,
                           src_core * K0 + src_off,
                           src_core * K1 + (src_off - RSPLIT))

    counts = np.zeros((NCORES, 2, NTILES), dtype=np.int64)
    buckets = {}
    for c in range(NCORES):
        mc = core_of == c
        for h in range(2):
            mh = mc & (half_of == h)
            for t in range(NTILES):
                m = mh & (tile_of == t)
                g = src_reg_idx[m]
                d = (dst[m] % NSH) % 128        # dst within tile
                w = invdeg_all[dst[m]]
                buckets[(c, h, t)] = (g, d, w)
                counts[c, h, t] = len(g)

    # global block schedule: both halves get >= 1 block per tile (pass-A copy
    # initializes agg; pass-B stop triggers the interleaved dense step)
    nblk = np.ceil(counts.max(axis=0) / BLK).astype(np.int64)  # [2, NTILES]
    nblk = np.maximum(nblk, 1)
    extra = [0, 0]
    for h in range(2):
        tot = int(nblk[h].sum())
        extra[h] = (-tot) % CBLK
    sched = []   # list of (h, t) per block, in execution order
    for h in range(2):
        for t in range(NTILES):
            sched += [(h, t)] * int(nblk[h, t])
        sched += [(h, NTILES - 1)] * extra[h]
    nblk_tot = len(sched)
    nchunks = nblk_tot // CBLK
    assert nchunks * CBLK == nblk_tot
    chunk_half = [sched[k * CBLK][0] for k in range(nchunks)]
    for k in range(nchunks):
        assert all(sched[k * CBLK + j][0] == chunk_half[k] for j in range(CBLK))

    # per-chunk gather pieces [(col offset in chunk, num_idxs)]
    run_start = {}
    b0 = 0
    for h in range(2):
        for t in range(NTILES):
            nb = int(nblk[h, t]) + (extra[h] if t == NTILES - 1 else 0)
            run_start[(h, t)] = (b0, nb)
            b0 += nb
    r16 = {k: min(-(-int(counts[:, k[0], k[1]].max()) // 16) * 16,
                  run_start[k][1] * BLK)
           for k in run_start}
    gather_pieces = []
    for k in range(nchunks):
        c0, c1 = k * CBLK * BLK, (k + 1) * CBLK * BLK
        iv = []
        for (h, t), (rb, nb) in run_start.items():
            if h != chunk_half[k]:
                continue
            s0, s1 = rb * BLK, rb * BLK + r16[(h, t)]
            a, b = max(s0, c0), min(s1, c1)
            if a < b:
                iv.append((a - c0, b - c0))
        pieces = []
        for q in range(0, CBLK * BLK, 1024):
            if any(a < q + 1024 and b > q for a, b in iv):
                pieces.append((q, 1024))
        gather_pieces.append(pieces)

    # per-core gather idx + per-block [dstidx, invdeg] following the schedule
    # (also original src node ids per slot, for the layer-0 host pregather)
    gidx_cores, div_cores, srcids_cores, S_cores = [], [], [], []
    src_orig = {}
    for c in range(NCORES):
        mc = core_of == c
        for h in range(2):
            mh = mc & (half_of == h)
            for t in range(NTILES):
                m = mh & (tile_of == t)
                src_orig[(c, h, t)] = src[m]
    for c in range(NCORES):
        gi = np.zeros(nblk_tot * BLK, dtype=np.int64)
        sid = np.zeros(nblk_tot * BLK, dtype=np.int64)
        div = np.zeros((nblk_tot * BLK, 2), dtype=np.float32)
        b0 = 0
        for h in range(2):
            for t in range(NTILES):
                nb = int(nblk[h, t]) + (extra[h] if t == NTILES - 1 else 0)
                g, d, w = buckets[(c, h, t)]
                n = len(g)
                gi[b0 * BLK: b0 * BLK + n] = g
                sid[b0 * BLK: b0 * BLK + n] = src_orig[(c, h, t)]
                div[b0 * BLK: b0 * BLK + n, 0] = d.astype(np.float32)
                div[b0 * BLK: b0 * BLK + n, 1] = w.astype(np.float32)
                b0 += nb
        assert b0 == nblk_tot
        gidx_cores.append(_wrap_idx(gi.astype(np.int16), CH))
        srcids_cores.append(sid)
        div_cores.append(np.ascontiguousarray(
            div.reshape(nblk_tot, BLK, 2).transpose(1, 0, 2)))
        Sm = np.zeros((nblk_tot * BLK, 128), dtype=np.float16)
        slots = np.arange(nblk_tot * BLK)
        Sm[slots, div[:, 0].astype(np.int64)] = div[:, 1].astype(np.float16)
        S_cores.append(np.ascontiguousarray(
            Sm.reshape(nblk_tot, BLK, 128).transpose(1, 0, 2)
            .reshape(BLK, nblk_tot * 128)))

    # last pass-B block index per tile (the interleaved dense trigger)
    pbstop = {}
    for b, (h, t) in enumerate(sched):
        if h == 1:
            pbstop[t] = b
    dense_after = {b: t for t, b in pbstop.items()}

    cnt_g = np.bincount(batch, minlength=G)
    inv_cnt = (1.0 / np.maximum(cnt_g, 1.0)).astype(np.float32)

    P = []
    for c in range(NCORES):
        p = np.zeros((NSH_PAD, G), dtype=np.float32)
        b = batch[c * NSH:(c + 1) * NSH]
        p[np.arange(NSH), b] = inv_cnt[b]
        P.append(p)

    NG, Smax = 0, 0
    groups_c = []
    for c in range(NCORES):
        b = batch[c * NSH:(c + 1) * NSH]
        glo, ghi = int(b.min()), int(b.max())
        groups = [(g, np.where(b == g)[0]) for g in range(glo, ghi + 1)]
        groups_c.append((glo, groups))
        NG = max(NG, ghi - glo + 1)
        Smax = max(Smax, max(len(gr) for _, gr in groups))
    S_slot = ((Smax + 127) // 128) * 128
    slot, route = [], []
    for c in range(NCORES):
        glo, groups = groups_c[c]
        sm = np.full(NG * S_slot, NSH, dtype=np.int64)   # NSH = zero dummy row
        R = np.zeros((NG, G), dtype=np.float32)
        for g, gr in groups:
            r = g - glo
            sm[r * S_slot:r * S_slot + len(gr)] = gr
            R[r, g] = 1.0
        slot.append(_wrap_idx(sm.astype(np.int16), NG * S_slot)[0])
        route.append(R)

    return dict(nblk=nblk, extra=extra, sched=sched, nblk_tot=nblk_tot,
                nchunks=nchunks, chunk_half=chunk_half,
                gather_pieces=gather_pieces, dense_after=dense_after,
                gidx=gidx_cores, div=div_cores, srcids=srcids_cores,
                S=S_cores, P=P,
                slot=slot, route=route, S_slot=S_slot, NG=NG)


# ---------------- device kernel builder --------------------------------------

def _build(nc, pre):
    import concourse.mybir as mybir
    import concourse.tile as tile

    f32 = mybir.dt.float32
    f16 = mybir.dt.float16
    i16 = mybir.dt.int16
    NCH = pre['nchunks']
    NBLK_TOT = pre['nblk_tot']
    NG, S_slot = pre['NG'], pre['S_slot']
    NSLOT = NG * S_slot
    sched = pre['sched']
    dense_after = pre['dense_after']

    is_start = [True] * NBLK_TOT
    is_stop = [True] * NBLK_TOT
    for b in range(NBLK_TOT):
        if b > 0 and sched[b] == sched[b - 1]:
            is_start[b] = False
        if b < NBLK_TOT - 1 and sched[b] == sched[b + 1]:
            is_stop[b] = False
    # block position within its run + run length (for A/B psum splitting)
    run_pos = [0] * NBLK_TOT
    run_len = [0] * NBLK_TOT
    p = 0
    for b in range(NBLK_TOT):
        run_pos[b] = p
        p = 0 if is_stop[b] else p + 1
    L = 0
    for b in range(NBLK_TOT - 1, -1, -1):
        if is_stop[b]:
            L = run_pos[b] + 1
        run_len[b] = L

    qctr = [0]

    def next_q():
        q = [1, 2, 3, 0][qctr[0] % 4]
        qctr[0] += 1
        return q

    # ---- I/O ----
    msgs0_d = nc.dram_tensor("msgs0", [NCH, 128, CBLK, D], f16,
                             kind="ExternalInput")
    S_d = nc.dram_tensor("S", [BLK, NBLK_TOT * 128], f16,
                         kind="ExternalInput")
    xownT = nc.dram_tensor("xownT", [128, NSH_PAD], f16, kind="ExternalInput")
    gidx_d = nc.dram_tensor("gidx", [NCH, 128, CPW], i16, kind="ExternalInput")
    div_d = nc.dram_tensor("div", [128, NBLK_TOT, 2], f32,
                           kind="ExternalInput")
    iota_d = nc.dram_tensor("iota", [128, 128], f16, kind="ExternalInput")
    slot_d = nc.dram_tensor("slot", [128, NSLOT // 16], i16, kind="ExternalInput")
    P_d = nc.dram_tensor("P", [NSH_PAD, G], f32, kind="ExternalInput")
    route_d = nc.dram_tensor("route", [NG, G], f32, kind="ExternalInput")
    gfT_d = nc.dram_tensor("gfT", [G_FEAT, G], f32, kind="ExternalInput")
    ident_d = nc.dram_tensor("ident", [128, 128], f32, kind="ExternalInput")
    Wl_d = [nc.dram_tensor(f"Wl{i}", [D, HID], f16, kind="ExternalInput")
            for i in range(3)]
    Wr_d = [nc.dram_tensor(f"Wr{i}", [D, HID], f16, kind="ExternalInput")
            for i in range(3)]
    gb_d = [nc.dram_tensor(f"gb{i}", [HID, 2], f32, kind="ExternalInput")
            for i in range(3)]
    W1_d = nc.dram_tensor("W1", [2 * HID + G_FEAT, HID], f32, kind="ExternalInput")
    W2_d = nc.dram_tensor("W2", [HID, HID // 2], f32, kind="ExternalInput")
    W3_d = nc.dram_tensor("W3", [HID // 2, 1], f32, kind="ExternalInput")
    bT_d = nc.dram_tensor("bT", [HID, 3], f32, kind="ExternalInput")

    out_d = nc.dram_tensor("out", [G], f32, kind="ExternalOutput")

    rg = [list(range(NCORES))]

    with tile.TileContext(nc) as tc:
        with (
            tc.tile_pool(name="sb", bufs=3) as sb,
            tc.tile_pool(name="big", bufs=2) as bigp,       # zT f16 ring
            tc.tile_pool(name="agg", bufs=1) as aggp,       # agg f16
            tc.tile_pool(name="big1", bufs=1) as big1,      # gmax/allp
            tc.tile_pool(name="msg", bufs=4) as msgp,
            tc.tile_pool(name="scache", bufs=1) as scp,
            tc.tile_pool(name="idx", bufs=4) as idxp,
            tc.tile_pool(name="cst", bufs=1) as cst,
            tc.tile_pool(name="ps", bufs=1, space="PSUM") as ps,
            tc.tile_pool(name="pst", bufs=2, space="PSUM") as pst,
            tc.tile_pool(name="psa", bufs=3, space="PSUM") as psa,
            tc.tile_pool(name="psm", bufs=1, space="PSUM") as psm,
            tc.tile_pool(name="dram", bufs=1, space="DRAM") as dram,
        ):
            # ---- DRAM scratch ----
            hbounce = [dram.tile([NSH_PAD, D], f16, tag=f"hb{i}", name=f"hb{i}")
                       for i in range(2)]
            tblA = [dram.tile([R0, D], f16, tag=f"tblA{i}", name=f"tblA{i}",
                              addr_space="Shared" if TABLE_SHARED else "Local")
                    for i in range(2)]
            tblB = [dram.tile([R1, D], f16, tag=f"tblB{i}", name=f"tblB{i}",
                              addr_space="Shared" if TABLE_SHARED else "Local")
                    for i in range(2)]
            h3bf = dram.tile([NSH + 128, D], f16, tag="h3bf")
            stats_in = [dram.tile([D, 2], f32, tag=f"stats_in{i}",
                                  name=f"stats_in{i}") for i in range(3)]
            stats_out = [dram.tile([NCORES * D, 2], f32, tag=f"stats_out{i}",
                                   name=f"stats_out{i}", addr_space="Shared")
                         for i in range(3)]
            pool_in = dram.tile([D, 2 * G], f32, tag="pool_in")
            pool_out = dram.tile([NCORES * D, 2 * G], f32, tag="pool_out",
                                 addr_space="Shared")

            def load_const(src_ap, rows, cols, name, dt=f32):
                t = cst.tile([rows, cols], dt, tag=name)
                nc.sync.dma_start(out=t[:, :], in_=src_ap)
                return t

            ident_sb = load_const(ident_d[:, :], 128, 128, "ident")
            ident16_sb = cst.tile([128, 128], f16, tag="ident16")
            nc.vector.tensor_copy(ident16_sb[:, :], ident_sb[:, :])
            iota_sb = load_const(iota_d[:, :], 128, 128, "iota", f16)
            div_sb = cst.tile([128, NBLK_TOT, 2], f32, tag="div")
            nc.sync.dma_start(out=div_sb[:, :, :], in_=div_d[:, :, :])
            xT_sb = cst.tile([128, NSH_PAD], f16, tag="xT")
            nc.sync.dma_start(out=xT_sb[:, :], in_=xownT[:, :])

            scache = [scp.tile([128, CBLK, 128], f16, tag=f"Sc{k}",
                               name=f"Sc{k}")
                      for k in range(NCACHE)]

            hT_prev = xT_sb

            for li in range(3):
                Wl_sb = load_const(Wl_d[li][:, :], D, HID, f"Wl{li}", f16)
                Wr_sb = load_const(Wr_d[li][:, :], D, HID, f"Wr{li}", f16)
                gb_sb = load_const(gb_d[li][:, :], HID, 2, f"gb{li}")

                agg_sb = aggp.tile([128, NSH_PAD], f16, tag="agg")
                zT = bigp.tile([128, NSH_PAD], f16, tag="zT")
                zsum = sb.tile([128, NTILES], f32, tag="zsum")
                zsq = sb.tile([128, NTILES], f32, tag="zsq")

                # ---- chunk loop: gather + S + agg matmuls + inline dense ----
                acc_ps = None
                for k in range(NCH):
                    h = pre['chunk_half'][k]
                    msgs = msgp.tile([128, CBLK, D], f16, tag="msgs")
                    if li == 0:
                        nc.sync.dma_start(out=msgs[:, :, :],
                                          in_=msgs0_d[k, :, :, :])
                    else:
                        src_tab = (tblA[(li - 1) % 2][:, :] if h == 0
                                   else tblB[(li - 1) % 2][:, :])
                        gi = idxp.tile([128, CPW], i16, tag="gi")
                        nc.sync.dma_start(out=gi[:], in_=gidx_d[k, :, :])
                        for off, n in pre['gather_pieces'][k]:
                            nc.gpsimd.dma_gather(
                                msgs[:, off // 128:
                                     off // 128 + (n + 127) // 128,
                                     :], src_tab,
                                gi[:, off // 16:off // 16 + n // 16], n, n, D,
                                queue_num=next_q())
                    if k < NCACHE:
                        S_sb = scache[k]
                    else:
                        S_sb = msgp.tile([128, CBLK, 128], f16, tag="Ssb")
                    if li == 0:
                        nc.sync.dma_start(
                            out=S_sb[:, :, :],
                            in_=S_d[:, k * CBLK * 128:(k + 1) * CBLK * 128]
                            .rearrange("p (j d) -> p j d", d=128))
                    for j in range(CBLK):
                        b = k * CBLK + j
                        h_b, t_b = sched[b]
                        if li > 0 and k >= NCACHE:
                            # S[slot, dstcol] = (iota == dstidx[slot])*invdeg
                            nc.vector.tensor_scalar(
                                S_sb[:, j, :], iota_sb[:, :],
                                div_sb[:, b, 0:1], div_sb[:, b, 1:2],
                                mybir.AluOpType.is_equal,
                                mybir.AluOpType.mult)
                        if is_start[b]:
                            acc_ps = psa.tile([128, D], f32, tag="accp")
                        nc.tensor.matmul(acc_ps[:, :], msgs[:, j, :],
                                         S_sb[:, j, :],
                                         start=is_start[b], stop=is_stop[b])
                        if is_stop[b]:
                            sl = agg_sb[:, t_b * 128:(t_b + 1) * 128]
                            if h_b == 0:
                                nc.scalar.copy(sl, acc_ps[:, :])
                            else:
                                nc.vector.tensor_add(sl, sl, acc_ps[:, :])
                            t_d = dense_after.get(b)
                            if t_d is not None:
                                z_ps = ps.tile([128, D], f32, tag="z")
                                nc.tensor.matmul(
                                    z_ps[:, :], Wl_sb[:, :],
                                    agg_sb[:, t_d * 128:(t_d + 1) * 128],
                                    start=True, stop=False)
                                nc.tensor.matmul(
                                    z_ps[:, :], Wr_sb[:, :],
                                    hT_prev[:, t_d * 128:(t_d + 1) * 128],
                                    start=False, stop=True)
                                nc.scalar.activation(
                                    zT[:, t_d * 128:(t_d + 1) * 128],
                                    z_ps[:, :],
                                    mybir.ActivationFunctionType.Copy,
                                    accum_out=zsum[:, t_d:t_d + 1])
                                sq_scr = sb.tile([128, D], f32, tag="sqscr")
                                nc.scalar.activation(
                                    sq_scr[:, :],
                                    zT[:, t_d * 128:(t_d + 1) * 128],
                                    mybir.ActivationFunctionType.Square,
                                    accum_out=zsq[:, t_d:t_d + 1])

                # ---- BN stats exchange + scale/shift ----
                stat_sb = sb.tile([128, 2], f32, tag="stat")
                nc.vector.tensor_reduce(stat_sb[:, 0:1], zsum[:, :],
                                        mybir.AxisListType.X,
                                        mybir.AluOpType.add)
                nc.vector.tensor_reduce(stat_sb[:, 1:2], zsq[:, :],
                                        mybir.AxisListType.X,
                                        mybir.AluOpType.add)
                nc.sync.dma_start(out=stats_in[li][:, :], in_=stat_sb[:, :])
                nc.gpsimd.collective_compute(
                    "AllGather", mybir.AluOpType.bypass, replica_groups=rg,
                    ins=[stats_in[li].opt()], outs=[stats_out[li].opt()])
                allst = sb.tile([128, NCORES, 2], f32, tag="allst")
                nc.sync.dma_start(
                    out=allst[:, :, :],
                    in_=stats_out[li][:, :].rearrange("(c p) j -> p c j",
                                                      c=NCORES))
                tot = sb.tile([128, 2], f32, tag="tot")
                nc.vector.tensor_add(tot[:, :], allst[:, 0, :], allst[:, 1, :])
                for c in range(2, NCORES):
                    nc.vector.tensor_add(tot[:, :], tot[:, :], allst[:, c, :])
                mu = sb.tile([128, 6], f32, tag="mu")
                nc.scalar.mul(mu[:, 0:1], tot[:, 0:1], 1.0 / N)
                nc.scalar.mul(mu[:, 1:2], tot[:, 1:2], 1.0 / N)
                nc.vector.tensor_mul(mu[:, 2:3], mu[:, 0:1], mu[:, 0:1])
                nc.vector.tensor_sub(mu[:, 3:4], mu[:, 1:2], mu[:, 2:3])
                nc.vector.tensor_scalar_add(mu[:, 3:4], mu[:, 3:4], EPS)
                nc.vector.reciprocal(mu[:, 4:5], mu[:, 3:4])
                nc.scalar.sqrt(mu[:, 4:5], mu[:, 4:5])
                nc.vector.tensor_mul(mu[:, 4:5], mu[:, 4:5], gb_sb[:, 0:1])
                nc.vector.tensor_mul(mu[:, 5:6], mu[:, 0:1], mu[:, 4:5])
                nc.vector.tensor_sub(mu[:, 5:6], gb_sb[:, 1:2], mu[:, 5:6])

                # ---- relu (real cols; pads stay 0) + per-tile transpose/ship
                if li < 2:
                    hb = hbounce[li % 2]
                    for ck in range(13):
                        w = 512 if ck < 12 else NSH - 12 * 512
                        nc.scalar.activation(zT[:, ck * 512:ck * 512 + w],
                                             zT[:, ck * 512:ck * 512 + w],
                                             mybir.ActivationFunctionType.Relu,
                                             bias=mu[:, 5:6], scale=mu[:, 4:5])
                        t0, t1 = ck * 4, min(ck * 4 + 4, NTILES)
                        for t in range(t0, t1):
                            hT_ps = pst.tile([128, D], f16, tag="tp16")
                            nc.tensor.transpose(
                                hT_ps[:, :], zT[:, t * 128:(t + 1) * 128],
                                ident16_sb[:, :])
                            hbf_sb = sb.tile([128, D], f16, tag="hbf")
                            nc.vector.tensor_copy(hbf_sb[:, :], hT_ps[:, :])
                            nc.sync.dma_start(
                                out=hb[t * 128:(t + 1) * 128, :],
                                in_=hbf_sb[:, :])
                        if t1 == 24:   # region 0 shipped -> AG0
                            nc.gpsimd.collective_compute(
                                "AllGather", mybir.AluOpType.bypass,
                                replica_groups=rg,
                                ins=[hb[0:RSPLIT, :].opt()],
                                outs=[tblA[li % 2].opt()])
                    nc.gpsimd.collective_compute(
                        "AllGather", mybir.AluOpType.bypass, replica_groups=rg,
                        ins=[hb[RSPLIT:NSH_PAD, :].opt()],
                        outs=[tblB[li % 2].opt()])
                else:
                    meanT_ps = psm.tile([128, G], f32, tag="meanT")
                    for ck in range(13):
                        w = 512 if ck < 12 else NSH - 12 * 512
                        nc.scalar.activation(zT[:, ck * 512:ck * 512 + w],
                                             zT[:, ck * 512:ck * 512 + w],
                                             mybir.ActivationFunctionType.Relu,
                                             bias=mu[:, 5:6], scale=mu[:, 4:5])
                        t0, t1 = ck * 4, min(ck * 4 + 4, NTILES)
                        for t in range(t0, t1):
                            hT_ps = pst.tile([128, D], f16, tag="tp16")
                            nc.tensor.transpose(
                                hT_ps[:, :], zT[:, t * 128:(t + 1) * 128],
                                ident16_sb[:, :])
                            h3_sb = sb.tile([128, D], f32, tag="h3")
                            nc.vector.tensor_copy(h3_sb[:, :], hT_ps[:, :])
                            P_sb = sb.tile([128, G], f32, tag="P")
                            nc.sync.dma_start(
                                out=P_sb[:, :],
                                in_=P_d[t * 128:(t + 1) * 128, :])
                            nc.tensor.matmul(meanT_ps[:, :], h3_sb[:, :],
                                             P_sb[:, :],
                                             start=(t == 0),
                                             stop=(t == NTILES - 1))
                            hbf_sb = sb.tile([128, D], f16, tag="hbf")
                            nc.vector.tensor_copy(hbf_sb[:, :], h3_sb[:, :])
                            nc.sync.dma_start(
                                out=h3bf[t * 128:(t + 1) * 128, :],
                                in_=hbf_sb[:, :])
                    zrow = sb.tile([1, D], f16, tag="zrow")
                    nc.vector.memset(zrow[:, :], 0.0)
                    nc.sync.dma_start(out=h3bf[NSH:NSH + 1, :], in_=zrow[:, :])
                hT_prev = zT

            # ---- max pool: transpose-gather + segmented max + route ----
            slot_sb = cst.tile([128, NSLOT // 16], i16, tag="slot")
            nc.sync.dma_start(out=slot_sb[:, :], in_=slot_d[:, :])
            SC = S_slot // 128
            gmax = big1.tile([128, NSLOT // 128, D], f16, tag="gmax")
            for g0 in range(0, NSLOT, 1024):
                g1 = min(g0 + 1024, NSLOT)
                nc.gpsimd.dma_gather(gmax[:, g0 // 128:g1 // 128, :],
                                     h3bf[0:NSH + 128, :],
                                     slot_sb[:, (g0 // 16):(g1 // 16)],
                                     g1 - g0, g1 - g0, D,
                                     queue_num=next_q())
            mloc_f = sb.tile([128, NG], f32, tag="mlocf")
            for r in range(NG):
                red1 = sb.tile([128, D], f32, tag="red1")
                nc.vector.tensor_reduce(
                    red1[:, :],
                    gmax[:, r * SC:(r + 1) * SC, :].rearrange("p c f -> p f c"),
                    mybir.AxisListType.X, mybir.AluOpType.max)
                r1T_ps = ps.tile([128, D], f32, tag="z", name="r1T")
                nc.tensor.transpose(r1T_ps[:, :], red1[:, :], ident_sb[:, :])
                r1T_sb = sb.tile([128, D], f32, tag="r1Ts")
                nc.vector.tensor_copy(r1T_sb[:, :], r1T_ps[:, :])
                nc.vector.tensor_reduce(mloc_f[:, r:r + 1], r1T_sb[:, :],
                                        mybir.AxisListType.X,
                                        mybir.AluOpType.max)
            mlocT_full = ps.tile([128, 128], f32, tag="z")
            mlocT_ps = mlocT_full[0:NG, :]
            nc.tensor.transpose(mlocT_ps, mloc_f[:, :], ident_sb[:, :])
            mlocT_sb = sb.tile([NG, 128], f32, tag="mlocTs")
            nc.vector.tensor_copy(mlocT_sb[:, :], mlocT_ps)
            route_sb = cst.tile([NG, G], f32, tag="route")
            nc.sync.dma_start(out=route_sb[:, :], in_=route_d[:, :])
            maxT_ps = psm.tile([128, G], f32, tag="tail")
            nc.tensor.matmul(maxT_ps[:, :], mlocT_sb[:, :], route_sb[:, :],
                             start=True, stop=True)

            # ---- pool partial exchange ----
            pool_sb = sb.tile([128, 2 * G], f32, tag="poolp")
            nc.vector.tensor_copy(pool_sb[:, 0:G], meanT_ps[:, :])
            nc.vector.tensor_copy(pool_sb[:, G:2 * G], maxT_ps[:, :])
            nc.sync.dma_start(out=pool_in[:, :], in_=pool_sb[:, :])
            nc.gpsimd.collective_compute(
                "AllGather", mybir.AluOpType.bypass, replica_groups=rg,
                ins=[pool_in.opt()], outs=[pool_out.opt()])
            allp = big1.tile([128, NCORES, 2 * G], f32, tag="allp")
            nc.sync.dma_start(
                out=allp[:, :, :],
                in_=pool_out[:, :].rearrange("(c p) j -> p c j", c=NCORES))
            meanTot = sb.tile([128, G], f32, tag="meanTot")
            maxTot = sb.tile([128, G], f32, tag="maxTot")
            nc.vector.tensor_add(meanTot[:, :], allp[:, 0, 0:G],
                                 allp[:, 1, 0:G])
            nc.vector.tensor_max(maxTot[:, :], allp[:, 0, G:2 * G],
                                 allp[:, 1, G:2 * G])
            for c in range(2, NCORES):
                nc.vector.tensor_add(meanTot[:, :], meanTot[:, :],
                                     allp[:, c, 0:G])
                nc.vector.tensor_max(maxTot[:, :], maxTot[:, :],
                                     allp[:, c, G:2 * G])

            # ---- head (feature-major) ----
            W1a_sb = load_const(W1_d[0:HID, :], HID, HID, "W1a")
            W1b_sb = load_const(W1_d[HID:2 * HID, :], HID, HID, "W1b")
            W1c_sb = load_const(W1_d[2 * HID:2 * HID + G_FEAT, :], G_FEAT,
                                HID, "W1c")
            W2_sb = load_const(W2_d[:, :], HID, HID // 2, "W2")
            W3_sb = load_const(W3_d[:, :], HID // 2, 1, "W3")
            bT_sb = load_const(bT_d[:, :], HID, 3, "bT")
            gfT_sb = load_const(gfT_d[:, :], G_FEAT, G, "gfT")

            m1_ps = psm.tile([HID, G], f32, tag="tail")
            nc.tensor.matmul(m1_ps[:, :], W1a_sb[:, :], meanTot[:, :],
                             start=True, stop=False)
            nc.tensor.matmul(m1_ps[:, :], W1b_sb[:, :], maxTot[:, :],
                             start=False, stop=False)
            nc.tensor.matmul(m1_ps[:, :], W1c_sb[:, :],
                             gfT_sb[:, :], start=False, stop=True)
            m1_sb = sb.tile([HID, G], f32, tag="m1s")
            nc.scalar.activation(m1_sb[:, :], m1_ps[:, :],
                                 mybir.ActivationFunctionType.Relu,
                                 bias=bT_sb[:, 0:1])
            m2_ps = psm.tile([HID // 2, G], f32, tag="tail")
            nc.tensor.matmul(m2_ps[:, :], W2_sb[:, :], m1_sb[:, :],
                             start=True, stop=True)
            m2_sb = sb.tile([HID // 2, G], f32, tag="m2s")
            nc.scalar.activation(m2_sb[:, :], m2_ps[:, :],
                                 mybir.ActivationFunctionType.Relu,
                                 bias=bT_sb[0:HID // 2, 1:2])
            m3_ps = psm.tile([1, G], f32, tag="tail")
            nc.tensor.matmul(m3_ps[:, :], W3_sb[:, :], m2_sb[:, :],
                             start=True, stop=True)
            m3_sb = sb.tile([1, G], f32, tag="m3s")
            nc.scalar.copy(m3_sb[:, :], m3_ps[:, :])
            nc.vector.tensor_scalar_add(m3_sb[:, :], m3_sb[:, :],
                                        bT_sb[0:1, 2:3])
            nc.sync.dma_start(out=out_d[:].rearrange("(o g) -> o g", o=1),
                              in_=m3_sb[:, :])
    return nc


# ---------------- public entry ------------------------------------------------

def build_in_maps(x, edge_index, batch, g_feats, params, pre):
    x = np.asarray(x, dtype=np.float32)
    g_feats = np.asarray(g_feats, dtype=np.float32)

    bT = np.zeros((HID, 3), np.float32)
    bT[:, 0] = np.asarray(params['b1'], np.float32)
    bT[:HID // 2, 1] = np.asarray(params['b2'], np.float32)
    bT[0, 2] = np.asarray(params['b3'], np.float32).reshape(-1)[0]

    iota = np.broadcast_to(np.arange(128, dtype=np.float16), (128, 128))

    x16 = x.astype(np.float16)

    common = {
        "iota": np.ascontiguousarray(iota),
        "ident": np.eye(128, dtype=np.float32),
        "gfT": np.ascontiguousarray(g_feats.T),
        "W1": np.asarray(params['W1'], np.float32),
        "W2": np.asarray(params['W2'], np.float32),
        "W3": np.asarray(params['W3'], np.float32),
        "bT": bT,
    }
    for i in range(3):
        common[f"Wl{i}"] = np.asarray(params[f'Wl{i}'],
                                      np.float32).astype(np.float16)
        common[f"Wr{i}"] = np.asarray(params[f'Wr{i}'],
                                      np.float32).astype(np.float16)
        gb = np.zeros((HID, 2), np.float32)
        gb[:, 0] = np.asarray(params[f'gamma{i}'], np.float32)
        gb[:, 1] = np.asarray(params[f'beta{i}'], np.float32)
        common[f"gb{i}"] = gb

    in_maps = []
    for c in range(NCORES):
        xo = np.zeros((NSH_PAD, D), np.float32)
        xo[:NSH] = x[c * NSH:(c + 1) * NSH]
        sid = pre['srcids'][c]
        m0 = x16[sid].reshape(-1, CBLK, 128, D).transpose(0, 2, 1, 3)
        m = dict(common)
        m.update({
            "msgs0": np.ascontiguousarray(m0),
            "S": pre['S'][c],
            "xownT": np.ascontiguousarray(xo.T).astype(np.float16),
            "gidx": pre['gidx'][c],
            "div": pre['div'][c],
            "slot": pre['slot'][c],
            "P": pre['P'][c],
            "route": pre['route'][c],
        })
        in_maps.append(m)
    return in_maps


def build_nc(pre):
    import concourse.bacc as bacc
    nc = bacc.Bacc(None, target_bir_lowering=False, debug=False,
                   num_devices=NCORES, num_swdge_queues=4,
                   dynamic_dma_scratch_size=24576)
    nc = _build(nc, pre)
    nc.compile()
    return nc


def kernel(x, edge_index, batch, g_feats,
           Wl0, bl0, Wr0, gamma0, beta0,
           Wl1, bl1, Wr1, gamma1, beta1,
           Wl2, bl2, Wr2, gamma2, beta2,
           W1, b1, W2, b2, W3, b3):
    # bl{i} cancels inside BatchNorm (constant pre-BN shift), so it is unused.
    from concourse.bass_utils import run_bass_kernel_spmd

    params = dict(Wl0=Wl0, Wr0=Wr0, gamma0=gamma0, beta0=beta0,
                  Wl1=Wl1, Wr1=Wr1, gamma1=gamma1, beta1=beta1,
                  Wl2=Wl2, Wr2=Wr2, gamma2=gamma2, beta2=beta2,
                  W1=W1, b1=b1, W2=W2, b2=b2, W3=W3, b3=b3)
    pre = _preprocess(x, edge_index, batch)
    nc = build_nc(pre)
    in_maps = build_in_maps(x, edge_index, batch, g_feats, params, pre)
    res = run_bass_kernel_spmd(nc, in_maps, list(range(NCORES)))
    return np.asarray(res.results[0]["out"], dtype=np.float32)


# revision 23
# speedup vs baseline: 1.1280x; 1.0934x over previous
"""Distributed Bass/Trainium2 kernel for nn_AreaGNN: 3x SAGEConv(mean) +
global BatchNorm + ReLU, per-graph mean/max pooling, 3-layer MLP head.
SPMD across 8 NeuronCores; takes FULL inputs, returns FULL output [G].

v5:
- dma_gather over 4 SWDGE queues (disjoint Q7 descgen pairs, ~4x).
- One-hot S blocks generated on-chip (DVE is_equal*invdeg) from a tiny
  per-block table; first NCACHE chunks cached in SBUF across layers.
- Aggregation matmul: out[feat, dst] = msgs^T @ S (feature-major agg, no
  dense-phase transposes); agg staged f16.
- Dense phase (z matmuls + BN stat accumulation) interleaved into the chunk
  loop per-tile as each tile's aggregation completes.
- Halo exchange split into TWO region AllGathers (shard rows 0:3072 and
  3072:6272). Gather tables are laid out region-major so region-0 chunks
  start as soon as AG0 lands while AG1 (and its transposes) overlap them.
"""
import numpy as np

N = 50000
E = 800000
D = 128
HID = 128
G = 64
G_FEAT = 32
EPS = 1e-5
NCORES = 8
NSH = N // NCORES           # 6250
NSH_PAD = 6272              # 49 * 128
NTILES = NSH_PAD // 128     # 49
RSPLIT = 3072               # region split within a shard (24 tiles | 25 tiles)
K0, K1 = RSPLIT, NSH_PAD - RSPLIT          # 3072, 3200 rows/core/region
R0, R1 = K0 * NCORES, K1 * NCORES          # 24576, 25600 table rows
BLK = 128                   # edges per S block
CBLK = 32                   # blocks per gather chunk (4096 edges)
CH = BLK * CBLK
CPW = CH // 16
NCACHE = 4                  # S chunks cached in SBUF across layers
TABLE_SHARED = True


# ---------------- host-side preprocessing -----------------------------------

def _wrap_idx(idx, ch):
    """[L] -> [L/ch, 128, ch/16] int16: element m of a chunk at (m%16, m//16),
    replicated across the eight 16-partition groups."""
    L = idx.shape[0]
    out = np.empty((L // ch, 128, ch // 16), dtype=np.int16)
    w = idx.reshape(L // ch, ch // 16, 16).transpose(0, 2, 1)
    for g in range(8):
        out[:, g * 16:(g + 1) * 16, :] = w
    return out


def _preprocess(x, edge_index, batch):
    src = np.asarray(edge_index[0], dtype=np.int64)
    dst = np.asarray(edge_index[1], dtype=np.int64)
    batch = np.asarray(batch, dtype=np.int64)

    indeg = np.bincount(dst, minlength=N)
    invdeg_all = (1.0 / np.maximum(indeg, 1.0)).astype(np.float32)

    core_of = dst // NSH
    tile_of = (dst % NSH) // 128
    # region of src within its owner shard + region-relative table index
    src_core = src // NSH
    src_off = src % NSH
    half_of = (src_off >= RSPLIT).astype(np.int64)
    src_reg_idx = np.where(half_of == 0,
                           src_core * K0 + src_off,
                           src_core * K1 + (src_off - RSPLIT))

    counts = np.zeros((NCORES, 2, NTILES), dtype=np.int64)
    buckets = {}
    for c in range(NCORES):
        mc = core_of == c
        for h in range(2):
            mh = mc & (half_of == h)
            for t in range(NTILES):
                m = mh & (tile_of == t)
                g = src_reg_idx[m]
                d = (dst[m] % NSH) % 128        # dst within tile
                w = invdeg_all[dst[m]]
                buckets[(c, h, t)] = (g, d, w)
                counts[c, h, t] = len(g)

    # global block schedule: both halves get >= 1 block per tile (pass-A copy
    # initializes agg; pass-B stop triggers the interleaved dense step)
    nblk = np.ceil(counts.max(axis=0) / BLK).astype(np.int64)  # [2, NTILES]
    nblk = np.maximum(nblk, 1)
    extra = [0, 0]
    for h in range(2):
        tot = int(nblk[h].sum())
        extra[h] = (-tot) % CBLK
    sched = []   # list of (h, t) per block, in execution order
    for h in range(2):
        for t in range(NTILES):
            sched += [(h, t)] * int(nblk[h, t])
        sched += [(h, NTILES - 1)] * extra[h]
    nblk_tot = len(sched)
    nchunks = nblk_tot // CBLK
    assert nchunks * CBLK == nblk_tot
    chunk_half = [sched[k * CBLK][0] for k in range(nchunks)]
    for k in range(nchunks):
        assert all(sched[k * CBLK + j][0] == chunk_half[k] for j in range(CBLK))

    # per-chunk gather pieces [(col offset in chunk, num_idxs)]
    run_start = {}
    b0 = 0
    for h in range(2):
        for t in range(NTILES):
            nb = int(nblk[h, t]) + (extra[h] if t == NTILES - 1 else 0)
            run_start[(h, t)] = (b0, nb)
            b0 += nb
    r16 = {k: min(-(-int(counts[:, k[0], k[1]].max()) // 16) * 16,
                  run_start[k][1] * BLK)
           for k in run_start}
    gather_pieces = []
    for k in range(nchunks):
        c0, c1 = k * CBLK * BLK, (k + 1) * CBLK * BLK
        iv = []
        for (h, t), (rb, nb) in run_start.items():
            if h != chunk_half[k]:
                continue
            s0, s1 = rb * BLK, rb * BLK + r16[(h, t)]
            a, b = max(s0, c0), min(s1, c1)
            if a < b:
                iv.append((a - c0, b - c0))
        pieces = []
        for q in range(0, CBLK * BLK, 1024):
            if any(a < q + 1024 and b > q for a, b in iv):
                pieces.append((q, 1024))
        gather_pieces.append(pieces)

    # per-core gather idx + per-block [dstidx, invdeg] following the schedule
    # (also original src node ids per slot, for the layer-0 host pregather)
    gidx_cores, div_cores, srcids_cores, S_cores = [], [], [], []
    src_orig = {}
    for c in range(NCORES):
        mc = core_of == c
        for h in range(2):
            mh = mc & (half_of == h)
            for t in range(NTILES):
                m = mh & (tile_of == t)
                src_orig[(c, h, t)] = src[m]
    for c in range(NCORES):
        gi = np.zeros(nblk_tot * BLK, dtype=np.int64)
        sid = np.zeros(nblk_tot * BLK, dtype=np.int64)
        div = np.zeros((nblk_tot * BLK, 2), dtype=np.float32)
        b0 = 0
        for h in range(2):
            for t in range(NTILES):
                nb = int(nblk[h, t]) + (extra[h] if t == NTILES - 1 else 0)
                g, d, w = buckets[(c, h, t)]
                n = len(g)
                gi[b0 * BLK: b0 * BLK + n] = g
                sid[b0 * BLK: b0 * BLK + n] = src_orig[(c, h, t)]
                div[b0 * BLK: b0 * BLK + n, 0] = d.astype(np.float32)
                div[b0 * BLK: b0 * BLK + n, 1] = w.astype(np.float32)
                b0 += nb
        assert b0 == nblk_tot
        gidx_cores.append(_wrap_idx(gi.astype(np.int16), CH))
        srcids_cores.append(sid)
        div_cores.append(np.ascontiguousarray(
            div.reshape(nblk_tot, BLK, 2).transpose(1, 0, 2)))
        Sm = np.zeros((nblk_tot * BLK, 128), dtype=np.float16)
        slots = np.arange(nblk_tot * BLK)
        Sm[slots, div[:, 0].astype(np.int64)] = div[:, 1].astype(np.float16)
        S_cores.append(np.ascontiguousarray(
            Sm.reshape(nblk_tot, BLK, 128).transpose(1, 0, 2)
            .reshape(BLK, nblk_tot * 128)))

    # last pass-B block index per tile (the interleaved dense trigger)
    pbstop = {}
    for b, (h, t) in enumerate(sched):
        if h == 1:
            pbstop[t] = b
    dense_after = {b: t for t, b in pbstop.items()}

    cnt_g = np.bincount(batch, minlength=G)
    inv_cnt = (1.0 / np.maximum(cnt_g, 1.0)).astype(np.float32)

    P = []
    for c in range(NCORES):
        p = np.zeros((NSH_PAD, G), dtype=np.float32)
        b = batch[c * NSH:(c + 1) * NSH]
        p[np.arange(NSH), b] = inv_cnt[b]
        P.append(p)

    NG, Smax = 0, 0
    groups_c = []
    for c in range(NCORES):
        b = batch[c * NSH:(c + 1) * NSH]
        glo, ghi = int(b.min()), int(b.max())
        groups = [(g, np.where(b == g)[0]) for g in range(glo, ghi + 1)]
        groups_c.append((glo, groups))
        NG = max(NG, ghi - glo + 1)
        Smax = max(Smax, max(len(gr) for _, gr in groups))
    S_slot = ((Smax + 127) // 128) * 128
    slot, route = [], []
    for c in range(NCORES):
        glo, groups = groups_c[c]
        sm = np.full(NG * S_slot, NSH, dtype=np.int64)   # NSH = zero dummy row
        R = np.zeros((NG, G), dtype=np.float32)
        for g, gr in groups:
            r = g - glo
            sm[r * S_slot:r * S_slot + len(gr)] = gr
            R[r, g] = 1.0
        slot.append(_wrap_idx(sm.astype(np.int16), NG * S_slot)[0])
        route.append(R)

    return dict(nblk=nblk, extra=extra, sched=sched, nblk_tot=nblk_tot,
                nchunks=nchunks, chunk_half=chunk_half,
                gather_pieces=gather_pieces, dense_after=dense_after,
                gidx=gidx_cores, div=div_cores, srcids=srcids_cores,
                S=S_cores, P=P,
                slot=slot, route=route, S_slot=S_slot, NG=NG)


# ---------------- device kernel builder --------------------------------------

def _build(nc, pre):
    import concourse.mybir as mybir
    import concourse.tile as tile

    f32 = mybir.dt.float32
    f16 = mybir.dt.float16
    i16 = mybir.dt.int16
    NCH = pre['nchunks']
    NBLK_TOT = pre['nblk_tot']
    NG, S_slot = pre['NG'], pre['S_slot']
    NSLOT = NG * S_slot
    sched = pre['sched']
    dense_after = pre['dense_after']

    is_start = [True] * NBLK_TOT
    is_stop = [True] * NBLK_TOT
    for b in range(NBLK_TOT):
        if b > 0 and sched[b] == sched[b - 1]:
            is_start[b] = False
        if b < NBLK_TOT - 1 and sched[b] == sched[b + 1]:
            is_stop[b] = False
    # block position within its run + run length (for A/B psum splitting)
    run_pos = [0] * NBLK_TOT
    run_len = [0] * NBLK_TOT
    p = 0
    for b in range(NBLK_TOT):
        run_pos[b] = p
        p = 0 if is_stop[b] else p + 1
    L = 0
    for b in range(NBLK_TOT - 1, -1, -1):
        if is_stop[b]:
            L = run_pos[b] + 1
        run_len[b] = L

    qctr = [0]

    def next_q():
        q = [1, 2, 3, 0][qctr[0] % 4]
        qctr[0] += 1
        return q

    # ---- I/O ----
    msgs0_d = nc.dram_tensor("msgs0", [NCH, 128, CBLK, D], f16,
                             kind="ExternalInput")
    S_d = nc.dram_tensor("S", [BLK, NBLK_TOT * 128], f16,
                         kind="ExternalInput")
    xownT = nc.dram_tensor("xownT", [128, NSH_PAD], f16, kind="ExternalInput")
    gidx_d = nc.dram_tensor("gidx", [NCH, 128, CPW], i16, kind="ExternalInput")
    div_d = nc.dram_tensor("div", [128, NBLK_TOT, 2], f32,
                           kind="ExternalInput")
    iota_d = nc.dram_tensor("iota", [128, 128], f16, kind="ExternalInput")
    slot_d = nc.dram_tensor("slot", [128, NSLOT // 16], i16, kind="ExternalInput")
    P_d = nc.dram_tensor("P", [NSH_PAD, G], f16, kind="ExternalInput")
    route_d = nc.dram_tensor("route", [NG, G], f32, kind="ExternalInput")
    gfT_d = nc.dram_tensor("gfT", [G_FEAT, G], f32, kind="ExternalInput")
    ident_d = nc.dram_tensor("ident", [128, 128], f32, kind="ExternalInput")
    Wl_d = [nc.dram_tensor(f"Wl{i}", [D, HID], f16, kind="ExternalInput")
            for i in range(3)]
    Wr_d = [nc.dram_tensor(f"Wr{i}", [D, HID], f16, kind="ExternalInput")
            for i in range(3)]
    gb_d = [nc.dram_tensor(f"gb{i}", [HID, 2], f32, kind="ExternalInput")
            for i in range(3)]
    W1_d = nc.dram_tensor("W1", [2 * HID + G_FEAT, HID], f32, kind="ExternalInput")
    W2_d = nc.dram_tensor("W2", [HID, HID // 2], f32, kind="ExternalInput")
    W3_d = nc.dram_tensor("W3", [HID // 2, 1], f32, kind="ExternalInput")
    bT_d = nc.dram_tensor("bT", [HID, 3], f32, kind="ExternalInput")

    out_d = nc.dram_tensor("out", [G], f32, kind="ExternalOutput")

    rg = [list(range(NCORES))]

    with tile.TileContext(nc) as tc:
        with (
            tc.tile_pool(name="sb", bufs=3) as sb,
            tc.tile_pool(name="big", bufs=2) as bigp,       # zT f16 ring
            tc.tile_pool(name="agg", bufs=1) as aggp,       # agg f16
            tc.tile_pool(name="big1", bufs=1) as big1,      # gmax/allp
            tc.tile_pool(name="msg", bufs=4) as msgp,
            tc.tile_pool(name="scache", bufs=1) as scp,
            tc.tile_pool(name="idx", bufs=4) as idxp,
            tc.tile_pool(name="cst", bufs=1) as cst,
            tc.tile_pool(name="ps", bufs=1, space="PSUM") as ps,
            tc.tile_pool(name="pst", bufs=2, space="PSUM") as pst,
            tc.tile_pool(name="psa", bufs=3, space="PSUM") as psa,
            tc.tile_pool(name="psm", bufs=1, space="PSUM") as psm,
            tc.tile_pool(name="dram", bufs=1, space="DRAM") as dram,
        ):
            # ---- DRAM scratch ----
            hbounce = [dram.tile([NSH_PAD, D], f16, tag=f"hb{i}", name=f"hb{i}")
                       for i in range(2)]
            tblA = [dram.tile([R0, D], f16, tag=f"tblA{i}", name=f"tblA{i}",
                              addr_space="Shared" if TABLE_SHARED else "Local")
                    for i in range(2)]
            tblB = [dram.tile([R1, D], f16, tag=f"tblB{i}", name=f"tblB{i}",
                              addr_space="Shared" if TABLE_SHARED else "Local")
                    for i in range(2)]
            h3bf = dram.tile([NSH + 128, D], f16, tag="h3bf")
            stats_in = [dram.tile([D, 2], f32, tag=f"stats_in{i}",
                                  name=f"stats_in{i}") for i in range(3)]
            stats_out = [dram.tile([NCORES * D, 2], f32, tag=f"stats_out{i}",
                                   name=f"stats_out{i}", addr_space="Shared")
                         for i in range(3)]
            pool_in = dram.tile([D, 2 * G], f32, tag="pool_in")
            pool_out = dram.tile([NCORES * D, 2 * G], f32, tag="pool_out",
                                 addr_space="Shared")

            def load_const(src_ap, rows, cols, name, dt=f32):
                t = cst.tile([rows, cols], dt, tag=name)
                nc.sync.dma_start(out=t[:, :], in_=src_ap)
                return t

            ident_sb = load_const(ident_d[:, :], 128, 128, "ident")
            ident16_sb = cst.tile([128, 128], f16, tag="ident16")
            nc.vector.tensor_copy(ident16_sb[:, :], ident_sb[:, :])
            iota_sb = load_const(iota_d[:, :], 128, 128, "iota", f16)
            div_sb = cst.tile([128, NBLK_TOT, 2], f32, tag="div")
            nc.sync.dma_start(out=div_sb[:, :, :], in_=div_d[:, :, :])
            xT_sb = cst.tile([128, NSH_PAD], f16, tag="xT")
            nc.sync.dma_start(out=xT_sb[:, :], in_=xownT[:, :])

            scache = [scp.tile([128, CBLK, 128], f16, tag=f"Sc{k}",
                               name=f"Sc{k}")
                      for k in range(NCACHE)]

            hT_prev = xT_sb

            for li in range(3):
                Wl_sb = load_const(Wl_d[li][:, :], D, HID, f"Wl{li}", f16)
                Wr_sb = load_const(Wr_d[li][:, :], D, HID, f"Wr{li}", f16)
                gb_sb = load_const(gb_d[li][:, :], HID, 2, f"gb{li}")

                agg_sb = aggp.tile([128, NSH_PAD], f16, tag="agg")
                zT = bigp.tile([128, NSH_PAD], f16, tag="zT")
                zsum = sb.tile([128, NTILES], f32, tag="zsum")
                zsq = sb.tile([128, NTILES], f32, tag="zsq")

                # ---- chunk loop: gather + S + agg matmuls + inline dense ----
                acc_ps = None
                for k in range(NCH):
                    h = pre['chunk_half'][k]
                    msgs = msgp.tile([128, CBLK, D], f16, tag="msgs")
                    if li == 0:
                        nc.sync.dma_start(out=msgs[:, :, :],
                                          in_=msgs0_d[k, :, :, :])
                    else:
                        src_tab = (tblA[(li - 1) % 2][:, :] if h == 0
                                   else tblB[(li - 1) % 2][:, :])
                        gi = idxp.tile([128, CPW], i16, tag="gi")
                        nc.sync.dma_start(out=gi[:], in_=gidx_d[k, :, :])
                        for pi, (off, n) in enumerate(
                                pre['gather_pieces'][k]):
                            nc.gpsimd.dma_gather(
                                msgs[:, off // 128:
                                     off // 128 + (n + 127) // 128,
                                     :], src_tab,
                                gi[:, off // 16:off // 16 + n // 16], n, n, D,
                                queue_num=[1, 2, 3, 0][pi])
                    if k < NCACHE:
                        S_sb = scache[k]
                    else:
                        S_sb = msgp.tile([128, CBLK, 128], f16, tag="Ssb")
                    if li == 0:
                        nc.sync.dma_start(
                            out=S_sb[:, :, :],
                            in_=S_d[:, k * CBLK * 128:(k + 1) * CBLK * 128]
                            .rearrange("p (j d) -> p j d", d=128))
                    for j in range(CBLK):
                        b = k * CBLK + j
                        h_b, t_b = sched[b]
                        if li > 0 and k >= NCACHE:
                            # S[slot, dstcol] = (iota == dstidx[slot])*invdeg
                            nc.vector.tensor_scalar(
                                S_sb[:, j, :], iota_sb[:, :],
                                div_sb[:, b, 0:1], div_sb[:, b, 1:2],
                                mybir.AluOpType.is_equal,
                                mybir.AluOpType.mult)
                        if is_start[b]:
                            acc_ps = psa.tile([128, D], f32, tag="accp")
                        nc.tensor.matmul(acc_ps[:, :], msgs[:, j, :],
                                         S_sb[:, j, :],
                                         start=is_start[b], stop=is_stop[b])
                        if is_stop[b]:
                            sl = agg_sb[:, t_b * 128:(t_b + 1) * 128]
                            if h_b == 0:
                                nc.scalar.copy(sl, acc_ps[:, :])
                            else:
                                nc.vector.tensor_add(sl, sl, acc_ps[:, :])
                            t_d = dense_after.get(b)
                            if t_d is not None:
                                z_ps = ps.tile([128, D], f32, tag="z")
                                nc.tensor.matmul(
                                    z_ps[:, :], Wl_sb[:, :],
                                    agg_sb[:, t_d * 128:(t_d + 1) * 128],
                                    start=True, stop=False)
                                nc.tensor.matmul(
                                    z_ps[:, :], Wr_sb[:, :],
                                    hT_prev[:, t_d * 128:(t_d + 1) * 128],
                                    start=False, stop=True)
                                nc.scalar.activation(
                                    zT[:, t_d * 128:(t_d + 1) * 128],
                                    z_ps[:, :],
                                    mybir.ActivationFunctionType.Copy,
                                    accum_out=zsum[:, t_d:t_d + 1])
                                sq_scr = sb.tile([128, D], f32, tag="sqscr")
                                nc.scalar.activation(
                                    sq_scr[:, :],
                                    zT[:, t_d * 128:(t_d + 1) * 128],
                                    mybir.ActivationFunctionType.Square,
                                    accum_out=zsq[:, t_d:t_d + 1])

                # ---- BN stats exchange + scale/shift ----
                stat_sb = sb.tile([128, 2], f32, tag="stat")
                nc.vector.tensor_reduce(stat_sb[:, 0:1], zsum[:, :],
                                        mybir.AxisListType.X,
                                        mybir.AluOpType.add)
                nc.vector.tensor_reduce(stat_sb[:, 1:2], zsq[:, :],
                                        mybir.AxisListType.X,
                                        mybir.AluOpType.add)
                nc.sync.dma_start(out=stats_in[li][:, :], in_=stat_sb[:, :])
                nc.gpsimd.collective_compute(
                    "AllGather", mybir.AluOpType.bypass, replica_groups=rg,
                    ins=[stats_in[li].opt()], outs=[stats_out[li].opt()])
                allst = sb.tile([128, NCORES, 2], f32, tag="allst")
                nc.sync.dma_start(
                    out=allst[:, :, :],
                    in_=stats_out[li][:, :].rearrange("(c p) j -> p c j",
                                                      c=NCORES))
                tot = sb.tile([128, 2], f32, tag="tot")
                nc.vector.tensor_add(tot[:, :], allst[:, 0, :], allst[:, 1, :])
                for c in range(2, NCORES):
                    nc.vector.tensor_add(tot[:, :], tot[:, :], allst[:, c, :])
                mu = sb.tile([128, 6], f32, tag="mu")
                nc.scalar.mul(mu[:, 0:1], tot[:, 0:1], 1.0 / N)
                nc.scalar.mul(mu[:, 1:2], tot[:, 1:2], 1.0 / N)
                nc.vector.tensor_mul(mu[:, 2:3], mu[:, 0:1], mu[:, 0:1])
                nc.vector.tensor_sub(mu[:, 3:4], mu[:, 1:2], mu[:, 2:3])
                nc.vector.tensor_scalar_add(mu[:, 3:4], mu[:, 3:4], EPS)
                nc.vector.reciprocal(mu[:, 4:5], mu[:, 3:4])
                nc.scalar.sqrt(mu[:, 4:5], mu[:, 4:5])
                nc.vector.tensor_mul(mu[:, 4:5], mu[:, 4:5], gb_sb[:, 0:1])
                nc.vector.tensor_mul(mu[:, 5:6], mu[:, 0:1], mu[:, 4:5])
                nc.vector.tensor_sub(mu[:, 5:6], gb_sb[:, 1:2], mu[:, 5:6])

                # ---- relu (real cols; pads stay 0) + per-tile transpose/ship
                if li < 2:
                    hb = hbounce[li % 2]
                    for ck in range(13):
                        w = 512 if ck < 12 else NSH - 12 * 512
                        nc.scalar.activation(zT[:, ck * 512:ck * 512 + w],
                                             zT[:, ck * 512:ck * 512 + w],
                                             mybir.ActivationFunctionType.Relu,
                                             bias=mu[:, 5:6], scale=mu[:, 4:5])
                        t0, t1 = ck * 4, min(ck * 4 + 4, NTILES)
                        for t in range(t0, t1):
                            hT_ps = pst.tile([128, D], f16, tag="tp16")
                            nc.tensor.transpose(
                                hT_ps[:, :], zT[:, t * 128:(t + 1) * 128],
                                ident16_sb[:, :])
                            hbf_sb = sb.tile([128, D], f16, tag="hbf")
                            nc.vector.tensor_copy(hbf_sb[:, :], hT_ps[:, :])
                            nc.sync.dma_start(
                                out=hb[t * 128:(t + 1) * 128, :],
                                in_=hbf_sb[:, :])
                        if t1 == 24:   # region 0 shipped -> AG0
                            nc.gpsimd.collective_compute(
                                "AllGather", mybir.AluOpType.bypass,
                                replica_groups=rg,
                                ins=[hb[0:RSPLIT, :].opt()],
                                outs=[tblA[li % 2].opt()])
                    nc.gpsimd.collective_compute(
                        "AllGather", mybir.AluOpType.bypass, replica_groups=rg,
                        ins=[hb[RSPLIT:NSH_PAD, :].opt()],
                        outs=[tblB[li % 2].opt()])
                else:
                    meanT_ps = psm.tile([128, G], f32, tag="meanT")
                    for ck in range(13):
                        w = 512 if ck < 12 else NSH - 12 * 512
                        nc.scalar.activation(zT[:, ck * 512:ck * 512 + w],
                                             zT[:, ck * 512:ck * 512 + w],
                                             mybir.ActivationFunctionType.Relu,
                                             bias=mu[:, 5:6], scale=mu[:, 4:5])
                        t0, t1 = ck * 4, min(ck * 4 + 4, NTILES)
                        for t in range(t0, t1):
                            hT_ps = pst.tile([128, D], f16, tag="tp16")
                            nc.tensor.transpose(
                                hT_ps[:, :], zT[:, t * 128:(t + 1) * 128],
                                ident16_sb[:, :])
                            hbf_sb = sb.tile([128, D], f16, tag="hbf")
                            nc.vector.tensor_copy(hbf_sb[:, :], hT_ps[:, :])
                            P_sb = sb.tile([128, G], f16, tag="P")
                            nc.sync.dma_start(
                                out=P_sb[:, :],
                                in_=P_d[t * 128:(t + 1) * 128, :])
                            nc.tensor.matmul(meanT_ps[:, :], hbf_sb[:, :],
                                             P_sb[:, :],
                                             start=(t == 0),
                                             stop=(t == NTILES - 1))
                            nc.sync.dma_start(
                                out=h3bf[t * 128:(t + 1) * 128, :],
                                in_=hbf_sb[:, :])
                    zrow = sb.tile([1, D], f16, tag="zrow")
                    nc.vector.memset(zrow[:, :], 0.0)
                    nc.sync.dma_start(out=h3bf[NSH:NSH + 1, :], in_=zrow[:, :])
                hT_prev = zT

            # ---- max pool: transpose-gather + segmented max + route ----
            slot_sb = cst.tile([128, NSLOT // 16], i16, tag="slot")
            nc.sync.dma_start(out=slot_sb[:, :], in_=slot_d[:, :])
            SC = S_slot // 128
            gmax = big1.tile([128, NSLOT // 128, D], f16, tag="gmax")
            for g0 in range(0, NSLOT, 1024):
                g1 = min(g0 + 1024, NSLOT)
                nc.gpsimd.dma_gather(gmax[:, g0 // 128:g1 // 128, :],
                                     h3bf[0:NSH + 128, :],
                                     slot_sb[:, (g0 // 16):(g1 // 16)],
                                     g1 - g0, g1 - g0, D,
                                     queue_num=next_q())
            mloc_f = sb.tile([128, NG], f32, tag="mlocf")
            for r in range(NG):
                red1 = sb.tile([128, D], f32, tag="red1")
                nc.vector.tensor_reduce(
                    red1[:, :],
                    gmax[:, r * SC:(r + 1) * SC, :].rearrange("p c f -> p f c"),
                    mybir.AxisListType.X, mybir.AluOpType.max)
                r1T_ps = ps.tile([128, D], f32, tag="z", name="r1T")
                nc.tensor.transpose(r1T_ps[:, :], red1[:, :], ident_sb[:, :])
                r1T_sb = sb.tile([128, D], f32, tag="r1Ts")
                nc.vector.tensor_copy(r1T_sb[:, :], r1T_ps[:, :])
                nc.vector.tensor_reduce(mloc_f[:, r:r + 1], r1T_sb[:, :],
                                        mybir.AxisListType.X,
                                        mybir.AluOpType.max)
            mlocT_full = ps.tile([128, 128], f32, tag="z")
            mlocT_ps = mlocT_full[0:NG, :]
            nc.tensor.transpose(mlocT_ps, mloc_f[:, :], ident_sb[:, :])
            mlocT_sb = sb.tile([NG, 128], f32, tag="mlocTs")
            nc.vector.tensor_copy(mlocT_sb[:, :], mlocT_ps)
            route_sb = cst.tile([NG, G], f32, tag="route")
            nc.sync.dma_start(out=route_sb[:, :], in_=route_d[:, :])
            maxT_ps = psm.tile([128, G], f32, tag="tail")
            nc.tensor.matmul(maxT_ps[:, :], mlocT_sb[:, :], route_sb[:, :],
                             start=True, stop=True)

            # ---- pool partial exchange ----
            pool_sb = sb.tile([128, 2 * G], f32, tag="poolp")
            nc.vector.tensor_copy(pool_sb[:, 0:G], meanT_ps[:, :])
            nc.vector.tensor_copy(pool_sb[:, G:2 * G], maxT_ps[:, :])
            nc.sync.dma_start(out=pool_in[:, :], in_=pool_sb[:, :])
            nc.gpsimd.collective_compute(
                "AllGather", mybir.AluOpType.bypass, replica_groups=rg,
                ins=[pool_in.opt()], outs=[pool_out.opt()])
            allp = big1.tile([128, NCORES, 2 * G], f32, tag="allp")
            nc.sync.dma_start(
                out=allp[:, :, :],
                in_=pool_out[:, :].rearrange("(c p) j -> p c j", c=NCORES))
            meanTot = sb.tile([128, G], f32, tag="meanTot")
            maxTot = sb.tile([128, G], f32, tag="maxTot")
            nc.vector.tensor_add(meanTot[:, :], allp[:, 0, 0:G],
                                 allp[:, 1, 0:G])
            nc.vector.tensor_max(maxTot[:, :], allp[:, 0, G:2 * G],
                                 allp[:, 1, G:2 * G])
            for c in range(2, NCORES):
                nc.vector.tensor_add(meanTot[:, :], meanTot[:, :],
                                     allp[:, c, 0:G])
                nc.vector.tensor_max(maxTot[:, :], maxTot[:, :],
                                     allp[:, c, G:2 * G])

            # ---- head (feature-major) ----
            W1a_sb = load_const(W1_d[0:HID, :], HID, HID, "W1a")
            W1b_sb = load_const(W1_d[HID:2 * HID, :], HID, HID, "W1b")
            W1c_sb = load_const(W1_d[2 * HID:2 * HID + G_FEAT, :], G_FEAT,
                                HID, "W1c")
            W2_sb = load_const(W2_d[:, :], HID, HID // 2, "W2")
            W3_sb = load_const(W3_d[:, :], HID // 2, 1, "W3")
            bT_sb = load_const(bT_d[:, :], HID, 3, "bT")
            gfT_sb = load_const(gfT_d[:, :], G_FEAT, G, "gfT")

            m1_ps = psm.tile([HID, G], f32, tag="tail")
            nc.tensor.matmul(m1_ps[:, :], W1a_sb[:, :], meanTot[:, :],
                             start=True, stop=False)
            nc.tensor.matmul(m1_ps[:, :], W1b_sb[:, :], maxTot[:, :],
                             start=False, stop=False)
            nc.tensor.matmul(m1_ps[:, :], W1c_sb[:, :],
                             gfT_sb[:, :], start=False, stop=True)
            m1_sb = sb.tile([HID, G], f32, tag="m1s")
            nc.scalar.activation(m1_sb[:, :], m1_ps[:, :],
                                 mybir.ActivationFunctionType.Relu,
                                 bias=bT_sb[:, 0:1])
            m2_ps = psm.tile([HID // 2, G], f32, tag="tail")
            nc.tensor.matmul(m2_ps[:, :], W2_sb[:, :], m1_sb[:, :],
                             start=True, stop=True)
            m2_sb = sb.tile([HID // 2, G], f32, tag="m2s")
            nc.scalar.activation(m2_sb[:, :], m2_ps[:, :],
                                 mybir.ActivationFunctionType.Relu,
                                 bias=bT_sb[0:HID // 2, 1:2])
            m3_ps = psm.tile([1, G], f32, tag="tail")
            nc.tensor.matmul(m3_ps[:, :], W3_sb[:, :], m2_sb[:, :],
                             start=True, stop=True)
            m3_sb = sb.tile([1, G], f32, tag="m3s")
            nc.scalar.copy(m3_sb[:, :], m3_ps[:, :])
            nc.vector.tensor_scalar_add(m3_sb[:, :], m3_sb[:, :],
                                        bT_sb[0:1, 2:3])
            nc.sync.dma_start(out=out_d[:].rearrange("(o g) -> o g", o=1),
                              in_=m3_sb[:, :])
    return nc


# ---------------- public entry ------------------------------------------------

def build_in_maps(x, edge_index, batch, g_feats, params, pre):
    x = np.asarray(x, dtype=np.float32)
    g_feats = np.asarray(g_feats, dtype=np.float32)

    bT = np.zeros((HID, 3), np.float32)
    bT[:, 0] = np.asarray(params['b1'], np.float32)
    bT[:HID // 2, 1] = np.asarray(params['b2'], np.float32)
    bT[0, 2] = np.asarray(params['b3'], np.float32).reshape(-1)[0]

    iota = np.broadcast_to(np.arange(128, dtype=np.float16), (128, 128))

    x16 = x.astype(np.float16)

    common = {
        "iota": np.ascontiguousarray(iota),
        "ident": np.eye(128, dtype=np.float32),
        "gfT": np.ascontiguousarray(g_feats.T),
        "W1": np.asarray(params['W1'], np.float32),
        "W2": np.asarray(params['W2'], np.float32),
        "W3": np.asarray(params['W3'], np.float32),
        "bT": bT,
    }
    for i in range(3):
        common[f"Wl{i}"] = np.asarray(params[f'Wl{i}'],
                                      np.float32).astype(np.float16)
        common[f"Wr{i}"] = np.asarray(params[f'Wr{i}'],
                                      np.float32).astype(np.float16)
        gb = np.zeros((HID, 2), np.float32)
        gb[:, 0] = np.asarray(params[f'gamma{i}'], np.float32)
        gb[:, 1] = np.asarray(params[f'beta{i}'], np.float32)
        common[f"gb{i}"] = gb

    in_maps = []
    for c in range(NCORES):
        xo = np.zeros((NSH_PAD, D), np.float32)
        xo[:NSH] = x[c * NSH:(c + 1) * NSH]
        sid = pre['srcids'][c]
        m0 = x16[sid].reshape(-1, CBLK, 128, D).transpose(0, 2, 1, 3)
        m = dict(common)
        m.update({
            "msgs0": np.ascontiguousarray(m0),
            "S": pre['S'][c],
            "xownT": np.ascontiguousarray(xo.T).astype(np.float16),
            "gidx": pre['gidx'][c],
            "div": pre['div'][c],
            "slot": pre['slot'][c],
            "P": pre['P'][c].astype(np.float16),
            "route": pre['route'][c],
        })
        in_maps.append(m)
    return in_maps


def build_nc(pre):
    import concourse.bacc as bacc
    nc = bacc.Bacc(None, target_bir_lowering=False, debug=False,
                   num_devices=NCORES, num_swdge_queues=4,
                   dynamic_dma_scratch_size=24576)
    nc = _build(nc, pre)
    nc.compile()
    return nc


def kernel(x, edge_index, batch, g_feats,
           Wl0, bl0, Wr0, gamma0, beta0,
           Wl1, bl1, Wr1, gamma1, beta1,
           Wl2, bl2, Wr2, gamma2, beta2,
           W1, b1, W2, b2, W3, b3):
    # bl{i} cancels inside BatchNorm (constant pre-BN shift), so it is unused.
    from concourse.bass_utils import run_bass_kernel_spmd

    params = dict(Wl0=Wl0, Wr0=Wr0, gamma0=gamma0, beta0=beta0,
                  Wl1=Wl1, Wr1=Wr1, gamma1=gamma1, beta1=beta1,
                  Wl2=Wl2, Wr2=Wr2, gamma2=gamma2, beta2=beta2,
                  W1=W1, b1=b1, W2=W2, b2=b2, W3=W3, b3=b3)
    pre = _preprocess(x, edge_index, batch)
    nc = build_nc(pre)
    in_maps = build_in_maps(x, edge_index, batch, g_feats, params, pre)
    res = run_bass_kernel_spmd(nc, in_maps, list(range(NCORES)))
    return np.asarray(res.results[0]["out"], dtype=np.float32)
